# revision 1
# baseline (speedup 1.0000x reference)
"""Trainium2 Bass kernel for nn_Attention (2-batch, 16-head, n=2048, d=64 causal
attention with LayerNorm-projected l2-normalized q/k, relative position bias,
and output projection), SPMD across 8 NeuronCores.

Sharding: launch A tensor-parallels the 16 heads (2 heads per core, both
batches on every core) and emits transposed attention outputs; launch B
row-shards the final @ Wo matmul across the 8 cores.

Matmuls run as bf16 (projections, stats, sim, attn@v) or float32r
(reduced-precision fp32, ~tf32 accuracy, full PE rate at N>=256; used where
rounding matters).  LayerNorm is folded into the projections: gamma folds into
the weights, the mean subtraction becomes a rank-1 matmul accumulation, and
rstd cancels in the q/k l2norm (applied to v as a cheap broadcast multiply).
Attention is computed in transposed layout simT[j, i] so the softmax
denominator falls out of the attn@v matmul via an appended ones-column on v
(row 64 of the output carries the denominators; launch B normalizes), and
causal masking is an affine_select on the diagonal bias tiles.
"""

import numpy as np

HEADS = 16
DH = 64
B = 2
N = 2048
DIM = 1024
EH = 128          # per-core slice of the inner dim (2 heads x 64)
NCORES = 8
IC = 512          # i-chunk width
NIC = N // IC     # 4 i-chunks
JT = 128          # j-tile width
NJT = N // JT     # 16 j-tiles
NCT = DIM // 128  # 8 contraction tiles
LN_EPS = 1e-5
NEG = -1e30

_cache = {}


def _build_launch_a():
    import concourse.bass as bass
    import concourse.tile as tile
    from concourse import bacc, mybir
    from concourse.masks import make_identity

    F32 = mybir.dt.float32
    F32R = mybir.dt.float32r
    BF16 = mybir.dt.bfloat16
    AF = mybir.ActivationFunctionType
    nc = bacc.Bacc(None)
    xT_d = nc.declare_dram_parameter("xT", [B, DIM, N], BF16, isOutput=False)
    rpbT_d = nc.declare_dram_parameter("rpbT", [2, N, N], F32, isOutput=False)
    wq_d = nc.declare_dram_parameter("wq", [DIM, EH], F32, isOutput=False)
    wk_d = nc.declare_dram_parameter("wk", [DIM, EH], F32, isOutput=False)
    wv_d = nc.declare_dram_parameter("wv", [DIM, EH], F32, isOutput=False)
    gamma_d = nc.declare_dram_parameter("gamma", [DIM], F32, isOutput=False)
    qs2_d = nc.declare_dram_parameter("qs2", [EH], F32, isOutput=False)
    ks2_d = nc.declare_dram_parameter("ks2", [EH], F32, isOutput=False)
    kb_d = nc.declare_dram_parameter("kb", [B, N], F32, isOutput=False)
    at_d = nc.declare_dram_parameter("at_out", [B, 2, 65, N], F32, isOutput=True)

    with tile.TileContext(nc) as tc:
        import contextlib
        with contextlib.ExitStack() as ctx:
            pers = ctx.enter_context(tc.tile_pool(name="pers", bufs=1))

            # ---------- constants ----------
            onescol_f = pers.tile([128, 1], F32, tag="onescol_f")
            nc.vector.memset(onescol_f, 1.0)
            onescol = pers.tile([128, 1], F32R, tag="onescol")
            nc.vector.tensor_copy(out=onescol, in_=onescol_f)
            onescol_bf = pers.tile([128, 1], BF16, tag="onescol_bf")
            nc.vector.tensor_copy(out=onescol_bf, in_=onescol_f)
            ones_row_bf = pers.tile([1, 128], BF16, tag="ones_row_bf")
            ident_bf = pers.tile([128, 128], BF16, tag="ident_bf")
            row_f = pers.tile([1, 128], F32, tag="row_f")
            nc.vector.memset(row_f, 1.0)
            ones_row = pers.tile([1, 128], F32R, tag="ones_row")
            nc.vector.tensor_copy(out=ones_row, in_=row_f)
            nc.vector.tensor_copy(out=ones_row_bf, in_=row_f)
            invn_f = pers.tile([1, 128], F32, tag="invn_f")
            nc.vector.memset(invn_f, 1.0 / DIM)
            invn_row = pers.tile([1, 128], F32R, tag="invn_row")
            nc.vector.tensor_copy(out=invn_row, in_=invn_f)
            o2_f = pers.tile([128, 2], F32, tag="o2_f")
            nc.vector.memset(o2_f, 0.0)
            nc.vector.memset(o2_f[0:64, 0:1], 1.0)
            nc.vector.memset(o2_f[64:128, 1:2], 1.0)
            ones2blk = pers.tile([128, 2], F32R, tag="ones2blk")
            nc.vector.tensor_copy(out=ones2blk, in_=o2_f)
            ident = pers.tile([128, 128], F32, tag="ident")
            make_identity(nc, ident)
            nc.vector.tensor_copy(out=ident_bf, in_=ident)
            eps128 = pers.tile([128, 1], F32, tag="eps128")
            nc.vector.memset(eps128, LN_EPS)
            eps2 = pers.tile([2, 1], F32, tag="eps2")
            nc.vector.memset(eps2, 1e-24)

            # scale rows -> block-diag [2, 128] (qs2blk[h, e] = qs2[e] iff head(e)==h)
            qsb_f = pers.tile([2, 128], F32, tag="qsb_f")
            nc.vector.memset(qsb_f, 0.0)
            nc.sync.dma_start(out=qsb_f[0:1, 0:64], in_=qs2_d.ap()[0:64].unsqueeze(0))
            nc.sync.dma_start(out=qsb_f[1:2, 64:128], in_=qs2_d.ap()[64:128].unsqueeze(0))
            qs2blk = pers.tile([2, 128], F32R, tag="qs2blk")
            nc.vector.tensor_copy(out=qs2blk, in_=qsb_f)
            ksb_f = pers.tile([2, 128], F32, tag="ksb_f")
            nc.vector.memset(ksb_f, 0.0)
            nc.sync.dma_start(out=ksb_f[0:1, 0:64], in_=ks2_d.ap()[0:64].unsqueeze(0))
            nc.sync.dma_start(out=ksb_f[1:2, 64:128], in_=ks2_d.ap()[64:128].unsqueeze(0))
            ks2blk = pers.tile([2, 128], F32R, tag="ks2blk")
            nc.vector.tensor_copy(out=ks2blk, in_=ksb_f)

            gam_sb = pers.tile([128, NCT], F32, tag="gam")
            nc.sync.dma_start(out=gam_sb, in_=gamma_d.ap().rearrange("(t p) -> p t", p=128))
            kbT = pers.tile([128, B, NJT], F32, tag="kbT")
            nc.sync.dma_start(out=kbT, in_=kb_d.ap().rearrange("b (t p) -> p b t", p=128))

            # ---------- weights: load, fold gamma, round to f32r ----------
            wps = {}
            css = {}
            with tc.tile_pool(name="wload", bufs=2) as wload, \
                 tc.tile_pool(name="cs_ps", bufs=1, space="PSUM") as cs_ps:
                for nm, wd in (("q", wq_d), ("k", wk_d), ("v", wv_d)):
                    wraw = wload.tile([128, NCT, EH], F32, tag="wraw")
                    nc.sync.dma_start(out=wraw, in_=wd.ap().rearrange("(t p) e -> p t e", p=128))
                    wp = pers.tile([128, NCT, EH], BF16, tag=f"w{nm}p")
                    for ct in range(NCT):
                        nc.vector.tensor_scalar_mul(
                            out=wp[:, ct, :], in0=wraw[:, ct, :], scalar1=gam_sb[:, ct:ct + 1])
                    cs = cs_ps.tile([1, EH], F32, tag="cs")
                    for ct in range(NCT):
                        nc.tensor.matmul(cs, onescol_bf, wp[:, ct, :],
                                         start=(ct == 0), stop=(ct == NCT - 1))
                    cs_sb = pers.tile([1, EH], BF16, tag=f"cs{nm}")
                    nc.vector.tensor_copy(out=cs_sb, in_=cs)
                    wps[nm] = wp
                    css[nm] = cs_sb

            # ---------- persistent per-batch products ----------
            qhat = [pers.tile([128, N], BF16, tag=f"qhat{b}", name=f"qhat{b}") for b in range(B)]
            khat = [pers.tile([128, N], BF16, tag=f"khat{b}", name=f"khat{b}") for b in range(B)]
            v_all = [pers.tile([128, NJT, 130], BF16, tag=f"vall{b}", name=f"vall{b}") for b in range(B)]

            for b in range(B):
                for jt in range(NJT):
                    nc.vector.tensor_copy(out=v_all[b][:, jt, 64:65], in_=onescol_f)
                    nc.vector.tensor_copy(out=v_all[b][:, jt, 129:130], in_=onescol_f)

            # ================= phase 1: LN stats + projections =================
            with tc.tile_pool(name="p1", bufs=2) as p1, \
                 tc.tile_pool(name="p1b", bufs=3) as p1b, \
                 tc.tile_pool(name="xr_pool", bufs=1) as xr_pool, \
                 tc.tile_pool(name="st_ps", bufs=1, space="PSUM") as st_ps, \
                 tc.tile_pool(name="pp_ps", bufs=4, space="PSUM") as pp_ps, \
                 tc.tile_pool(name="bc_ps", bufs=2, space="PSUM") as bc_ps:
                for b in range(B):
                    xr = xr_pool.tile([128, NCT, N], BF16, tag="xr")
                    for half in range(2):
                        hs = slice(half * (NCT // 2), (half + 1) * (NCT // 2))
                        nc.sync.dma_start(
                            out=xr[:, hs, :],
                            in_=xT_d.ap()[b, half * 512:(half + 1) * 512, :].rearrange(
                                "(t p) n -> p t n", p=128))

                    numu = pers.tile([1, N], BF16, tag=f"numu{b}")
                    for ic in range(NIC):
                        isl = slice(ic * IC, (ic + 1) * IC)
                        # --- stats ---
                        sx = st_ps.tile([1, IC], F32, tag="sx")
                        sxx = st_ps.tile([1, IC], F32, tag="sxx")
                        for ct in range(NCT):
                            nc.tensor.matmul(sx, onescol_bf, xr[:, ct, isl],
                                             start=(ct == 0), stop=(ct == NCT - 1))
                        for ct in range(NCT):
                            x2 = p1b.tile([128, IC], BF16, tag="tmpb")
                            nc.vector.tensor_mul(x2, xr[:, ct, isl], xr[:, ct, isl])
                            nc.tensor.matmul(sxx, onescol_bf, x2,
                                             start=(ct == 0), stop=(ct == NCT - 1))
                        nc.scalar.mul(out=numu[:, isl], in_=sx, mul=-1.0 / DIM)
                        sxx_sb = p1b.tile([1, IC], F32R, tag="rowtmp")
                        nc.vector.tensor_copy(out=sxx_sb, in_=sxx)
                        # broadcast stats to 128 partitions via K=1 matmuls
                        mub = bc_ps.tile([128, IC], F32, tag="bc")
                        nc.tensor.matmul(mub, ones_row_bf, numu[:, isl], start=True, stop=True)
                        sxxb = bc_ps.tile([128, IC], F32, tag="bc")
                        nc.tensor.matmul(sxxb, invn_row, sxx_sb, start=True, stop=True)
                        mu2 = p1b.tile([128, IC], F32, tag="tmp")
                        nc.scalar.activation(out=mu2, in_=mub, func=AF.Square)
                        var = p1b.tile([128, IC], F32, tag="tmp")
                        nc.vector.tensor_sub(var, sxxb, mu2)
                        std = p1b.tile([128, IC], F32, tag="tmp")
                        nc.scalar.activation(out=std, in_=var, func=AF.Sqrt, bias=eps128)
                        rstd = p1b.tile([128, IC], F32, tag="rstd", bufs=2)
                        nc.vector.reciprocal_approx_fast(out=rstd, in_=std)

                        # --- q/k projections + l2norm ---
                        for nm, hat, sblk, sq_scale in (
                            ("q", qhat[b], qs2blk, 1.0 / 64.0),
                            ("k", khat[b], ks2blk, 1.0),
                        ):
                            pp = pp_ps.tile([128, IC], F32, tag="proj")
                            for ct in range(NCT):
                                nc.tensor.matmul(pp, wps[nm][:, ct, :], xr[:, ct, isl],
                                                 start=(ct == 0), stop=False)
                            nc.tensor.matmul(pp, css[nm], numu[:, isl], start=False, stop=True)
                            sq = p1b.tile([128, IC], F32R, tag="tmp")
                            nc.scalar.activation(out=sq, in_=pp, func=AF.Square)
                            ssq = bc_ps.tile([2, IC], F32, tag="bc")
                            nc.tensor.matmul(ssq, ones2blk, sq, start=True, stop=True)
                            rt = p1b.tile([2, IC], F32, tag="rowtmp")
                            nc.scalar.activation(out=rt, in_=ssq, func=AF.Sqrt,
                                                 bias=eps2, scale=sq_scale)
                            rn_f = p1b.tile([2, IC], F32, tag="rowtmp")
                            nc.vector.reciprocal_approx_fast(out=rn_f, in_=rt)
                            rn = p1b.tile([2, IC], F32R, tag="rowtmp")
                            nc.vector.tensor_copy(out=rn, in_=rn_f)
                            sr = bc_ps.tile([128, IC], F32, tag="bc")
                            nc.tensor.matmul(sr, sblk, rn, start=True, stop=True)
                            sr_sb = p1b.tile([128, IC], F32, tag="srsb")
                            nc.vector.tensor_copy(out=sr_sb, in_=sr)
                            nc.vector.tensor_mul(hat[:, isl], pp, sr_sb)

                        # --- v projection (rstd applied), transpose to [j, e] ---
                        vp = pp_ps.tile([128, IC], F32, tag="proj")
                        for ct in range(NCT):
                            nc.tensor.matmul(vp, wps["v"][:, ct, :], xr[:, ct, isl],
                                             start=(ct == 0), stop=False)
                        nc.tensor.matmul(vp, css["v"], numu[:, isl], start=False, stop=True)
                        vsc = p1b.tile([128, IC], BF16, tag="tmpb")
                        nc.vector.tensor_mul(vsc, vp, rstd)
                        for k in range(IC // 128):
                            jt = ic * (IC // 128) + k
                            vt = bc_ps.tile([128, 128], BF16, tag="bc")
                            nc.tensor.transpose(vt, vsc[:, k * 128:(k + 1) * 128], ident_bf)
                            nc.vector.tensor_copy(out=v_all[b][:, jt, 0:64], in_=vt[:, 0:64])
                            nc.vector.tensor_copy(out=v_all[b][:, jt, 65:129], in_=vt[:, 64:128])

            # ================= phase 2: attention =================
            with tc.tile_pool(name="rp_pool", bufs=4) as rp_pool, \
                 tc.tile_pool(name="rpd_pool", bufs=8) as rpd_pool, \
                 tc.tile_pool(name="es_pool", bufs=5) as es_pool, \
                 tc.tile_pool(name="E_pool", bufs=4) as E_pool, \
                 tc.tile_pool(name="at_pool", bufs=2) as at_pool, \
                 tc.tile_pool(name="sm_ps", bufs=4, space="PSUM") as sm_ps, \
                 tc.tile_pool(name="av_ps", bufs=2, space="PSUM") as av_ps:
                for ic in range(NIC):
                    isl = slice(ic * IC, (ic + 1) * IC)
                    jmax = (IC // 128) * (ic + 1)
                    rps = []
                    rpds = []
                    for h in range(2):
                        rp = rp_pool.tile([128, NJT, IC], F32, tag="rp", name=f"rp{h}")
                        nc.sync.dma_start(
                            out=rp[:, 0:jmax, :],
                            in_=rpbT_d.ap()[h, 0:jmax * 128, isl].rearrange(
                                "(t p) i -> p t i", p=128))
                        rpd = []
                        for k in range(IC // 128):
                            jt = jmax - (IC // 128) + k
                            rd = rpd_pool.tile([128, IC], F32, tag="rpd", name=f"rpd{h}{k}")
                            nc.gpsimd.affine_select(
                                out=rd, in_=rp[:, jt, :],
                                compare_op=mybir.AluOpType.is_ge,
                                fill=NEG, base=-128 * k, channel_multiplier=-1,
                                pattern=[[1, IC]])
                            rpd.append(rd)
                        rps.append(rp)
                        rpds.append(rpd)
                    for b in range(B):
                        avs = [av_ps.tile([65, IC], F32, tag=f"av{h}", name=f"av{h}")
                               for h in range(2)]
                        for jg in range(0, jmax, 2):
                            sps = {}
                            for jt in range(jg, min(jg + 2, jmax)):
                                for h in range(2):
                                    dsl = slice(64 * h, 64 * h + 64)
                                    sp = sm_ps.tile([128, IC], F32, tag="sim", name="sp")
                                    nc.tensor.matmul(
                                        sp, khat[b][dsl, jt * 128:(jt + 1) * 128],
                                        qhat[b][dsl, isl], start=True, stop=True)
                                    sps[jt, h] = sp
                            for jt in range(jg, min(jg + 2, jmax)):
                                for h in range(2):
                                    diag_k = jt - (jmax - (IC // 128))
                                    bias_tile = rpds[h][diag_k] if diag_k >= 0 else rps[h][:, jt, :]
                                    es = es_pool.tile([128, IC], F32, tag="es")
                                    nc.vector.tensor_add(es, sps[jt, h], bias_tile)
                                    E = E_pool.tile([128, IC], BF16, tag="E")
                                    nc.scalar.activation(out=E, in_=es, func=AF.Exp,
                                                         bias=kbT[:, b, jt:jt + 1])
                                    nc.tensor.matmul(
                                        avs[h], v_all[b][:, jt, 65 * h:65 * h + 65], E,
                                        start=(jt == 0), stop=(jt == jmax - 1))
                        for h in range(2):
                            stg = at_pool.tile([65, IC], F32, tag="stg")
                            nc.vector.tensor_copy(out=stg, in_=avs[h][0:65, :])
                            nc.sync.dma_start(out=at_d.ap()[b, h, :, isl], in_=stg)
    nc.compile()
    return nc


def _build_launch_b():
    import concourse.bass as bass
    import concourse.tile as tile
    from concourse import bacc, mybir

    F32 = mybir.dt.float32
    F32R = mybir.dt.float32r
    BF16 = mybir.dt.bfloat16

    nc = bacc.Bacc(None)
    at_d = nc.declare_dram_parameter("a_t", [DIM, IC], F32, isOutput=False)
    s_d = nc.declare_dram_parameter("s_slice", [HEADS, IC], F32, isOutput=False)
    sel_d = nc.declare_dram_parameter("sel", [NCT, HEADS, 128], F32, isOutput=False)
    wo_d = nc.declare_dram_parameter("wo", [DIM, DIM], F32, isOutput=False)
    out_d = nc.declare_dram_parameter("out_rows", [IC, DIM], F32, isOutput=True)

    with tile.TileContext(nc) as tc:
        with tc.tile_pool(name="sb", bufs=1) as sb, \
             tc.tile_pool(name="wl", bufs=2) as wl, \
             tc.tile_pool(name="ob", bufs=2) as ob, \
             tc.tile_pool(name="rb_ps", bufs=2, space="PSUM") as rb_ps, \
             tc.tile_pool(name="ps", bufs=2, space="PSUM") as ps:
            a_sb = sb.tile([128, NCT, IC], F32, tag="a")
            for half in range(2):
                hs = slice(half * (NCT // 2), (half + 1) * (NCT // 2))
                nc.sync.dma_start(
                    out=a_sb[:, hs, :],
                    in_=at_d.ap()[half * 512:(half + 1) * 512, :].rearrange(
                        "(t p) i -> p t i", p=128))
            s_sb = sb.tile([HEADS, IC], F32, tag="s")
            nc.sync.dma_start(out=s_sb, in_=s_d.ap())
            sel_sb = sb.tile([HEADS, NCT, 128], F32, tag="sel")
            nc.sync.dma_start(out=sel_sb, in_=sel_d.ap().rearrange("t h p -> h t p"))
            rs_f = sb.tile([HEADS, IC], F32, tag="rs_f")
            nc.vector.reciprocal_approx_fast(out=rs_f, in_=s_sb)
            rs_r = sb.tile([HEADS, IC], F32R, tag="rs_r")
            nc.vector.tensor_copy(out=rs_r, in_=rs_f)
            wo_r = sb.tile([128, NCT, DIM], F32R, tag="wo")
            for ct in range(NCT):
                wr = wl.tile([128, DIM], F32, tag="wr")
                nc.sync.dma_start(out=wr, in_=wo_d.ap()[ct * 128:(ct + 1) * 128, :])
                nc.vector.tensor_copy(out=wo_r[:, ct, :], in_=wr)
            # normalized bf16 activations: a_n[c, i] = a[c, i] / s[head(c), i]
            a_n = sb.tile([128, NCT, IC], F32R, tag="a_n")
            for ct in range(NCT):
                selr = wl.tile([HEADS, 128], F32R, tag="selr")
                nc.vector.tensor_copy(out=selr, in_=sel_sb[:, ct, :])
                rsb = rb_ps.tile([128, IC], F32, tag="rsb")
                nc.tensor.matmul(rsb, selr, rs_r, start=True, stop=True)
                nc.vector.tensor_mul(a_n[:, ct, :], rsb, a_sb[:, ct, :])
            for m in range(IC // 128):
                osb = ob.tile([128, DIM], F32, tag="osb")
                for oc in range(2):
                    pp = ps.tile([128, 512], F32, tag="pp")
                    for ct in range(NCT):
                        nc.tensor.matmul(
                            pp, a_n[:, ct, m * 128:(m + 1) * 128],
                            wo_r[:, ct, oc * 512:(oc + 1) * 512],
                            start=(ct == 0), stop=(ct == NCT - 1))
                    nc.vector.tensor_copy(out=osb[:, oc * 512:(oc + 1) * 512], in_=pp)
                nc.sync.dma_start(out=out_d.ap()[m * 128:(m + 1) * 128, :], in_=osb)

    nc.compile()
    return nc


PROFILE = {"enabled": False, "a_ns": None, "b_ns": None}


def _install_profile_hook():
    """Register the axon NTFF profile hook (the image's antenv lacks
    axon_hooks, so run_bass_kernel_spmd(trace=True) would silently skip
    tracing).  Replicates trn_boot's ctypes recipe."""
    import sys, types, ctypes, contextlib

    if "antenv.axon_hooks" in sys.modules:
        return
    lib = ctypes.CDLL("/opt/axon/libaxon_pjrt.so")
    if not hasattr(lib, "axon_start_nrt_profile"):
        return
    lib.axon_start_nrt_profile.argtypes = [ctypes.POINTER(ctypes.c_int64), ctypes.c_size_t]
    lib.axon_start_nrt_profile.restype = ctypes.c_int64
    lib.axon_stop_nrt_profile.argtypes = [ctypes.c_char_p]
    lib.axon_stop_nrt_profile.restype = ctypes.c_int64

    @contextlib.contextmanager
    def _hook(output_dir, device_ids):
        import jax
        jax.devices()
        if device_ids:
            ids = (ctypes.c_int64 * len(device_ids))(*device_ids)
            rc = lib.axon_start_nrt_profile(ids, len(device_ids))
        else:
            rc = lib.axon_start_nrt_profile(None, 0)
        if rc != 0:
            raise RuntimeError(f"axon_start_nrt_profile rc={rc}")
        try:
            yield
        finally:
            n = lib.axon_stop_nrt_profile(str(output_dir).encode())
            print(f"profile: {n} file(s) written to {output_dir}")

    mod = types.ModuleType("antenv.axon_hooks")
    mod.get_axon_ntff_profile_hook = lambda: _hook
    mod.set_axon_ntff_profile_hook = lambda h: None
    sys.modules["antenv.axon_hooks"] = mod

    # avoid the S3 artifact upload inside the trace path
    from concourse import bass_utils
    bass_utils.upload_artifacts = lambda tmpdir: ""


def kernel(x, gamma, Wq, Wkv, q_scale, k_scale, Wo, rel_pos_bias, mask):
    from concourse.bass_utils import run_bass_kernel_spmd

    x = np.ascontiguousarray(np.asarray(x, dtype=np.float32))
    gamma = np.asarray(gamma, dtype=np.float32)
    Wq = np.asarray(Wq, dtype=np.float32)
    Wkv = np.asarray(Wkv, dtype=np.float32)
    q_scale = np.asarray(q_scale, dtype=np.float32)
    k_scale = np.asarray(k_scale, dtype=np.float32)
    Wo = np.ascontiguousarray(np.asarray(Wo, dtype=np.float32))
    rel_pos_bias = np.asarray(rel_pos_bias, dtype=np.float32)
    mask = np.asarray(mask)

    if PROFILE["enabled"]:
        _install_profile_hook()
    if "a" not in _cache:
        _cache["a"] = _build_launch_a()
    if "b" not in _cache:
        _cache["b"] = _build_launch_b()

    import ml_dtypes
    xT = np.ascontiguousarray(x.transpose(0, 2, 1)).astype(ml_dtypes.bfloat16)
    kb = np.where(mask, 0.0, NEG).astype(np.float32)
    qs2 = np.tile(q_scale, 2).astype(np.float32)
    ks2 = np.tile(k_scale, 2).astype(np.float32)

    in_maps_a = []
    for c in range(NCORES):
        es = slice(EH * c, EH * (c + 1))
        in_maps_a.append({
            "xT": xT,
            "rpbT": np.ascontiguousarray(rel_pos_bias[2 * c:2 * c + 2].transpose(0, 2, 1)),
            "wq": np.ascontiguousarray(Wq[:, es]),
            "wk": np.ascontiguousarray(Wkv[:, :DIM][:, es]),
            "wv": np.ascontiguousarray(Wkv[:, DIM:][:, es]),
            "gamma": gamma, "qs2": qs2, "ks2": ks2, "kb": kb,
        })
    res_a = run_bass_kernel_spmd(_cache["a"], in_maps_a, list(range(NCORES)),
                                 trace=PROFILE["enabled"])
    if PROFILE["enabled"]:
        PROFILE["a_ns"] = res_a.exec_time_ns

    AT = np.empty((B, DIM, N), np.float32)
    S = np.empty((B, HEADS, N), np.float32)
    for c in range(NCORES):
        ao = res_a.results[c]["at_out"]            # [B, 2, 65, N]
        for h in range(2):
            AT[:, EH * c + 64 * h:EH * c + 64 * h + 64, :] = ao[:, h, 0:64, :]
            S[:, 2 * c + h, :] = ao[:, h, 64, :]

    sel = np.zeros((NCT, HEADS, 128), np.float32)
    for ct in range(NCT):
        sel[ct, 2 * ct, 0:64] = 1.0
        sel[ct, 2 * ct + 1, 64:128] = 1.0

    in_maps_b = []
    for c in range(NCORES):
        bi, ic = c // NIC, c % NIC
        in_maps_b.append({
            "a_t": np.ascontiguousarray(AT[bi][:, ic * IC:(ic + 1) * IC]),
            "s_slice": np.ascontiguousarray(S[bi][:, ic * IC:(ic + 1) * IC]),
            "sel": sel,
            "wo": Wo,
        })
    res_b = run_bass_kernel_spmd(_cache["b"], in_maps_b, list(range(NCORES)),
                                 trace=PROFILE["enabled"])
    if PROFILE["enabled"]:
        PROFILE["b_ns"] = res_b.exec_time_ns

    out = np.empty((B, N, DIM), np.float32)
    for c in range(NCORES):
        bi, ic = c // NIC, c % NIC
        out[bi, ic * IC:(ic + 1) * IC, :] = res_b.results[c]["out_rows"]
    return out



# revision 6
# speedup vs baseline: 1.2435x; 1.2435x over previous
"""Trainium2 Bass kernel for nn_Attention (2-batch, 16-head, n=2048, d=64 causal
attention with LayerNorm-projected l2-normalized q/k, relative position bias,
and output projection), SPMD across 8 NeuronCores.

Sharding: launch A tensor-parallels the 16 heads (2 heads per core, both
batches on every core) and emits transposed attention outputs; launch B
row-shards the final @ Wo matmul across the 8 cores.

Key structure (v2):
 - LayerNorm stats (mean/var) computed on host; gamma folded into the
   projection weights on host; the mean subtraction is a rank-1 matmul
   accumulation; rstd cancels in q/k l2norm and is applied to v.
 - rel_pos_bias enters multiplicatively: host precomputes B = exp(bias^T)
   in bf16, device computes E = exp(sim) straight out of PSUM (one wide
   activation over 4 PSUM banks = 2 j-tiles x 2 heads), then E*B on
   DVE/GpSimd in bf16.  Causal masking = affine_select fill 0.0 on B.
 - sim matmuls for the 2 heads are emitted as adjacent row-tiled (K=64)
   pairs at PE tile positions (0,0)/(64,0) so they can overlap.
 - attn@v uses a 65-wide v||ones stationary; row 64 carries softmax
   denominators; launch B normalizes and row-shards @ Wo in bf16.
 - phase 1 of batch 1 is software-pipelined into phase 2 of batch 0 to
   keep the tensor engine busy during the Act-bound softmax stretches.
"""

import numpy as np

HEADS = 16
DH = 64
B = 2
N = 2048
DIM = 1024
EH = 128          # per-core slice of the inner dim (2 heads x 64)
NCORES = 8
IC = 512          # i-chunk width
NIC = N // IC     # 4 i-chunks
JT = 128          # j-tile width
NJT = N // JT     # 16 j-tiles
NCT = DIM // 128  # 8 contraction tiles
LN_EPS = 1e-5
NEG = -1e30

_cache = {}


def _build_launch_a(mask_ones=True):
    import concourse.bass as bass
    import concourse.tile as tile
    from concourse import bacc, mybir
    from concourse.masks import make_identity

    F32 = mybir.dt.float32
    F32R = mybir.dt.float32r
    BF16 = mybir.dt.bfloat16
    AF = mybir.ActivationFunctionType
    nc = bacc.Bacc(None)
    xT_d = nc.declare_dram_parameter("xT", [B, DIM, N], BF16, isOutput=False)
    rpbE_d = nc.declare_dram_parameter("rpbE", [NJT, 2, 128, N], BF16, isOutput=False)
    wq_d = nc.declare_dram_parameter("wq", [DIM, EH], BF16, isOutput=False)
    wk_d = nc.declare_dram_parameter("wk", [DIM, EH], BF16, isOutput=False)
    wv_d = nc.declare_dram_parameter("wv", [DIM, EH], BF16, isOutput=False)
    csq_d = nc.declare_dram_parameter("csq", [1, EH], BF16, isOutput=False)
    csk_d = nc.declare_dram_parameter("csk", [1, EH], BF16, isOutput=False)
    csv_d = nc.declare_dram_parameter("csv", [1, EH], BF16, isOutput=False)
    numu_d = nc.declare_dram_parameter("numu", [B, N], BF16, isOutput=False)
    rstd_d = nc.declare_dram_parameter("rstd", [B, N], BF16, isOutput=False)
    sbq_d = nc.declare_dram_parameter("sblk4q", [4, 128], F32, isOutput=False)
    sbk_d = nc.declare_dram_parameter("sblk4k", [4, 128], F32, isOutput=False)
    if not mask_ones:
        kb_d = nc.declare_dram_parameter("kb", [B, N], F32, isOutput=False)
    at_d = nc.declare_dram_parameter("at_out", [B, 2, 65, N], F32, isOutput=True)

    with tile.TileContext(nc) as tc:
        import contextlib
        with contextlib.ExitStack() as ctx:
            pers = ctx.enter_context(tc.tile_pool(name="pers", bufs=1))

            # ---------- constants ----------
            onescol_f = pers.tile([128, 1], F32, tag="onescol_f")
            nc.vector.memset(onescol_f, 1.0)
            row_f = pers.tile([1, 128], F32, tag="row_f")
            nc.vector.memset(row_f, 1.0)
            ones_row_bf = pers.tile([1, 128], BF16, tag="ones_row_bf")
            nc.vector.tensor_copy(out=ones_row_bf, in_=row_f)
            ident = pers.tile([128, 128], F32, tag="ident")
            make_identity(nc, ident)
            ident_bf = pers.tile([128, 128], BF16, tag="ident_bf")
            nc.vector.tensor_copy(out=ident_bf, in_=ident)
            eps4 = pers.tile([4, 1], F32, tag="eps4")
            nc.vector.memset(eps4, 1e-24)

            # ssq stationaries: o4q cols 0-1 head-blockdiag, o4k cols 2-3
            o4_f = pers.tile([128, 4], F32, tag="o4_f")
            nc.vector.memset(o4_f, 0.0)
            nc.vector.memset(o4_f[0:64, 0:1], 1.0)
            nc.vector.memset(o4_f[64:128, 1:2], 1.0)
            o4q = pers.tile([128, 4], BF16, tag="o4q")
            nc.vector.tensor_copy(out=o4q, in_=o4_f)
            nc.vector.memset(o4_f, 0.0)
            nc.vector.memset(o4_f[0:64, 2:3], 1.0)
            nc.vector.memset(o4_f[64:128, 3:4], 1.0)
            o4k = pers.tile([128, 4], BF16, tag="o4k")
            nc.vector.tensor_copy(out=o4k, in_=o4_f)

            # scale-broadcast stationaries (f32r)
            sbq_f = pers.tile([4, 128], F32, tag="sbq_f")
            nc.sync.dma_start(out=sbq_f, in_=sbq_d.ap())
            sbq_r = pers.tile([4, 128], F32R, tag="sbq_r")
            nc.vector.tensor_copy(out=sbq_r, in_=sbq_f)
            sbk_f = pers.tile([4, 128], F32, tag="sbk_f")
            nc.sync.dma_start(out=sbk_f, in_=sbk_d.ap())
            sbk_r = pers.tile([4, 128], F32R, tag="sbk_r")
            nc.vector.tensor_copy(out=sbk_r, in_=sbk_f)

            # weights (host gamma-folded), correction rows, LN stats rows
            wps = {}
            css = {}
            for nm, wd, cd in (("q", wq_d, csq_d), ("k", wk_d, csk_d),
                               ("v", wv_d, csv_d)):
                wp = pers.tile([128, NCT, EH], BF16, tag=f"w{nm}p", name=f"wp{nm}")
                nc.sync.dma_start(out=wp, in_=wd.ap().rearrange("(t p) e -> p t e", p=128))
                cs = pers.tile([1, EH], BF16, tag=f"cs{nm}", name=f"cs{nm}")
                nc.sync.dma_start(out=cs, in_=cd.ap())
                wps[nm] = wp
                css[nm] = cs
            numu_sb = pers.tile([1, B, N], BF16, tag="numu_sb")
            nc.sync.dma_start(out=numu_sb, in_=numu_d.ap().unsqueeze(0))
            rstd_sb = pers.tile([1, B, N], BF16, tag="rstd_sb")
            nc.sync.dma_start(out=rstd_sb, in_=rstd_d.ap().unsqueeze(0))
            if not mask_ones:
                kbT = pers.tile([128, B, NJT], F32, tag="kbT")
                nc.sync.dma_start(out=kbT, in_=kb_d.ap().rearrange("b (t p) -> p b t", p=128))

            # persistent per-batch products
            qhat = [pers.tile([128, N], BF16, tag=f"qhat{b}", name=f"qhat{b}") for b in range(B)]
            khat = [pers.tile([128, N], BF16, tag=f"khat{b}", name=f"khat{b}") for b in range(B)]
            v_all = [pers.tile([128, NJT, 130], BF16, tag=f"vall{b}", name=f"vall{b}") for b in range(B)]
            for b in range(B):
                for jt in range(NJT):
                    nc.vector.tensor_copy(out=v_all[b][:, jt, 64:65], in_=onescol_f)
                    nc.vector.tensor_copy(out=v_all[b][:, jt, 129:130], in_=onescol_f)

            # ---------- pools ----------
            sim_ps = ctx.enter_context(tc.tile_pool(name="sim_ps", bufs=1, space="PSUM"))
            av_ps = ctx.enter_context(tc.tile_pool(name="av_ps", bufs=2, space="PSUM"))
            mix_ps = ctx.enter_context(tc.tile_pool(name="mix_ps", bufs=2, space="PSUM"))
            xr_pool = ctx.enter_context(tc.tile_pool(name="xr_pool", bufs=2))
            bc_pool = ctx.enter_context(tc.tile_pool(name="bc_pool", bufs=2))
            bm_pool = ctx.enter_context(tc.tile_pool(name="bm_pool", bufs=2))
            e_pool = ctx.enter_context(tc.tile_pool(name="e_pool", bufs=2))
            m_pool = ctx.enter_context(tc.tile_pool(name="m_pool", bufs=2))
            sq_pool = ctx.enter_context(tc.tile_pool(name="sq_pool", bufs=2))
            rn_pool = ctx.enter_context(tc.tile_pool(name="rn_pool", bufs=2))
            raw_pool = ctx.enter_context(tc.tile_pool(name="raw_pool", bufs=2))
            vsc_pool = ctx.enter_context(tc.tile_pool(name="vsc_pool", bufs=2))
            stg_pool = ctx.enter_context(tc.tile_pool(name="stg_pool", bufs=2))

            # ---------- phase-1 work units ----------
            def ph1_units(b):
                """Emission closures for LN+proj+l2norm of one (b, ic) chunk,
                chunk-major.  Each unit is ~0.5-2us of tensor work."""
                units = []
                state = {}
                for ic in range(NIC):
                    isl = slice(ic * IC, (ic + 1) * IC)

                    def u_load(b=b, ic=ic, isl=isl):
                        xr = xr_pool.tile([128, NCT, IC], BF16, tag="xr", name="xr")
                        nc.sync.dma_start(
                            out=xr,
                            in_=xT_d.ap()[b, :, isl].rearrange("(t p) n -> p t n", p=128))
                        state[ic] = {"xr": xr}
                    units.append(u_load)

                    def mk_proj(nm, b=b, ic=ic, isl=isl):
                        def u_proj_a():
                            st = state[ic]
                            pp = mix_ps.tile([128, IC], F32, tag="mx", name=f"pp{nm}")
                            for ct in range(4):
                                nc.tensor.matmul(pp, wps[nm][:, ct, :], st["xr"][:, ct, :],
                                                 start=(ct == 0), stop=False)
                            st[f"pp{nm}"] = pp
                        def u_proj_b():
                            st = state[ic]
                            pp = st[f"pp{nm}"]
                            for ct in range(4, NCT):
                                nc.tensor.matmul(pp, wps[nm][:, ct, :], st["xr"][:, ct, :],
                                                 start=False, stop=False)
                            nc.tensor.matmul(pp, css[nm], numu_sb[0:1, b, isl],
                                             start=False, stop=True)
                        return u_proj_a, u_proj_b
                    qa, qb = mk_proj("q")
                    ka, kb_ = mk_proj("k")
                    va, vb = mk_proj("v")

                    def u_qpost(b=b, ic=ic):
                        st = state[ic]
                        q_raw = raw_pool.tile([128, IC], BF16, tag="raw", name="q_raw")
                        nc.vector.tensor_copy(out=q_raw, in_=st["ppq"])
                        sq_q = sq_pool.tile([128, IC], BF16, tag="sq", name="sq_q")
                        nc.vector.tensor_mul(sq_q, q_raw, q_raw)
                        st["q_raw"] = q_raw
                        st["sq_q"] = sq_q

                    def u_kpost(b=b, ic=ic):
                        st = state[ic]
                        k_raw = raw_pool.tile([128, IC], BF16, tag="raw", name="k_raw")
                        nc.vector.tensor_copy(out=k_raw, in_=st["ppk"])
                        sq_k = sq_pool.tile([128, IC], BF16, tag="sq", name="sq_k")
                        nc.vector.tensor_mul(sq_k, k_raw, k_raw)
                        st["k_raw"] = k_raw
                        st["sq_k"] = sq_k

                    def u_rn(b=b, ic=ic):
                        st = state[ic]
                        ssq4 = mix_ps.tile([4, IC], F32, tag="mx", name="ssq4")
                        nc.tensor.matmul(ssq4, o4q, st["sq_q"], start=True, stop=False)
                        nc.tensor.matmul(ssq4, o4k, st["sq_k"], start=False, stop=True)
                        ln4 = rn_pool.tile([4, IC], F32, tag="rn", name="ln4")
                        nc.scalar.activation(out=ln4, in_=ssq4, func=AF.Ln, bias=eps4)
                        rn4 = rn_pool.tile([4, IC], F32, tag="rn", name="rn4")
                        nc.scalar.activation(out=rn4, in_=ln4, func=AF.Exp, scale=-0.5)
                        rn4r = rn_pool.tile([4, IC], F32R, tag="rnr", name="rn4r")
                        nc.vector.tensor_copy(out=rn4r, in_=rn4)
                        st["rn4r"] = rn4r

                    def u_hats(b=b, ic=ic, isl=isl):
                        st = state[ic]
                        sr_q = mix_ps.tile([128, IC], F32, tag="mx", name="sr_q")
                        nc.tensor.matmul(sr_q, sbq_r, st["rn4r"], start=True, stop=True)
                        nc.vector.tensor_mul(qhat[b][:, isl], st["q_raw"], sr_q)
                        sr_k = mix_ps.tile([128, IC], F32, tag="mx", name="sr_k")
                        nc.tensor.matmul(sr_k, sbk_r, st["rn4r"], start=True, stop=True)
                        nc.vector.tensor_mul(khat[b][:, isl], st["k_raw"], sr_k)

                    def u_vpost(b=b, ic=ic, isl=isl):
                        st = state[ic]
                        rstdb = mix_ps.tile([128, IC], F32, tag="mx", name="rstdb")
                        nc.tensor.matmul(rstdb, ones_row_bf, rstd_sb[0:1, b, isl],
                                         start=True, stop=True)
                        rb_sb = vsc_pool.tile([128, IC], F32, tag="rb", name="rb_sb")
                        nc.vector.tensor_copy(out=rb_sb, in_=rstdb)
                        vsc = vsc_pool.tile([128, IC], BF16, tag="vsc", name="vsc")
                        nc.vector.tensor_mul(vsc, st["ppv"], rb_sb)
                        st["vsc"] = vsc

                    def u_vtrans(b=b, ic=ic):
                        st = state[ic]
                        for k in range(IC // 128):
                            jt = ic * (IC // 128) + k
                            vt = mix_ps.tile([128, 128], BF16, tag="mx", name="vt")
                            nc.tensor.transpose(vt, st["vsc"][:, k * 128:(k + 1) * 128], ident_bf)
                            nc.vector.tensor_copy(out=v_all[b][:, jt, 0:64], in_=vt[:, 0:64])
                            nc.vector.tensor_copy(out=v_all[b][:, jt, 65:129], in_=vt[:, 64:128])
                        del state[ic]

                    units += [qa, qb, u_qpost, ka, kb_, u_kpost, u_rn,
                              u_hats, va, vb, u_vpost, u_vtrans]
                return units

            # ---------- phase-2 (attention) ----------
            def load_bias_chunk(ic):
                jmax = (IC // 128) * (ic + 1)
                isl = slice(ic * IC, (ic + 1) * IC)
                Bc = bc_pool.tile([128, NJT, 2, IC], BF16, tag="bc", name="Bc")
                nc.sync.dma_start(
                    out=Bc[:, 0:jmax, :, :],
                    in_=rpbE_d.ap()[0:jmax, :, :, isl].rearrange("t h p i -> p t h i"))
                Bm = bm_pool.tile([128, 4, 2, IC], BF16, tag="bm", name="Bm")
                for k in range(4):
                    for h in range(2):
                        nc.gpsimd.affine_select(
                            out=Bm[:, k, h, :], in_=Bc[:, jmax - 4 + k, h, :],
                            compare_op=mybir.AluOpType.is_ge,
                            fill=0.0, base=-128 * k, channel_multiplier=-1,
                            pattern=[[1, IC]])
                return Bc, Bm

            def ph2_chunk(b, ic, Bc, Bm, feed):
                """feed: list of ph1 unit closures to interleave between groups."""
                jmax = (IC // 128) * (ic + 1)
                isl = slice(ic * IC, (ic + 1) * IC)
                avs = [av_ps.tile([65, IC], F32, tag="av", name=f"av{h}")
                       for h in range(2)]
                for g in range(jmax // 2):
                    jt0 = 2 * g
                    sp = sim_ps.tile([128, 2, 2, IC], F32, tag="sp", name="sp")
                    for l in range(2):
                        jt = jt0 + l
                        for h in range(2):
                            dsl = slice(64 * h, 64 * h + 64)
                            nc.tensor.matmul(
                                sp[:, l, h, :],
                                khat[b][dsl, jt * 128:(jt + 1) * 128],
                                qhat[b][dsl, isl], start=True, stop=True)
                    E = e_pool.tile([128, 2, 2, IC], BF16, tag="E", name="E")
                    if mask_ones:
                        nc.scalar.activation(out=E, in_=sp, func=AF.Exp)
                    else:
                        for l in range(2):
                            jt = jt0 + l
                            for h in range(2):
                                nc.scalar.activation(out=E[:, l, h, :],
                                                     in_=sp[:, l, h, :],
                                                     func=AF.Exp,
                                                     bias=kbT[:, b, jt:jt + 1])
                    Em = m_pool.tile([128, 2, 2, IC], BF16, tag="Em", name="Em")
                    diag0 = jmax - 4
                    if jt0 >= diag0:
                        bsrc = Bm[:, jt0 - diag0:jt0 - diag0 + 2, :, :]
                    else:
                        bsrc = Bc[:, jt0:jt0 + 2, :, :]
                    eng = nc.vector if (g % 2 == 0) else nc.gpsimd
                    eng.tensor_mul(Em, E, bsrc)
                    for l in range(2):
                        jt = jt0 + l
                        for h in range(2):
                            nc.tensor.matmul(
                                avs[h], v_all[b][:, jt, 65 * h:65 * h + 65],
                                Em[:, l, h, :],
                                start=(jt == 0), stop=(jt == jmax - 1))
                    # software-pipeline phase-1 work of the other batch
                    for _ in range(2):
                        if feed:
                            feed.pop(0)()
                for h in range(2):
                    stg = stg_pool.tile([65, IC], F32, tag="stg", name="stg")
                    nc.vector.tensor_copy(out=stg, in_=avs[h][0:65, :])
                    nc.sync.dma_start(out=at_d.ap()[b, h, :, isl], in_=stg)

            # ---------- main schedule ----------
            for u in ph1_units(0):
                u()
            feed = ph1_units(1)
            UPC = len(feed) // NIC          # units per chunk
            for ic in range(NIC):
                Bc, Bm = load_bias_chunk(ic)
                ph2_chunk(0, ic, Bc, Bm, feed)
                # ph2(b1, ic) needs ph1(b1) chunks 0..ic complete: drain until
                # only the units of chunks > ic remain
                while feed and len(feed) > UPC * (NIC - 1 - ic):
                    feed.pop(0)()
                ph2_chunk(1, ic, Bc, Bm, [])
    nc.compile()
    return nc


def _build_launch_b():
    import concourse.bass as bass
    import concourse.tile as tile
    from concourse import bacc, mybir

    F32 = mybir.dt.float32
    F32R = mybir.dt.float32r
    BF16 = mybir.dt.bfloat16

    nc = bacc.Bacc(None)
    at_d = nc.declare_dram_parameter("a_t", [DIM, IC], BF16, isOutput=False)
    s_d = nc.declare_dram_parameter("s_slice", [HEADS, IC], F32, isOutput=False)
    sel_d = nc.declare_dram_parameter("sel", [NCT, HEADS, 128], F32, isOutput=False)
    wo_d = nc.declare_dram_parameter("wo", [DIM, DIM], BF16, isOutput=False)
    out_d = nc.declare_dram_parameter("out_rows", [IC, DIM], BF16, isOutput=True)

    with tile.TileContext(nc) as tc:
        with tc.tile_pool(name="sb", bufs=1) as sb, \
             tc.tile_pool(name="wl", bufs=2) as wl, \
             tc.tile_pool(name="ob", bufs=2) as ob, \
             tc.tile_pool(name="rb_ps", bufs=2, space="PSUM") as rb_ps, \
             tc.tile_pool(name="ps", bufs=2, space="PSUM") as ps:
            a_sb = sb.tile([128, NCT, IC], BF16, tag="a")
            nc.sync.dma_start(
                out=a_sb,
                in_=at_d.ap().rearrange("(t p) i -> p t i", p=128))
            s_sb = sb.tile([HEADS, IC], F32, tag="s")
            nc.sync.dma_start(out=s_sb, in_=s_d.ap())
            sel_sb = sb.tile([HEADS, NCT, 128], F32, tag="sel")
            nc.sync.dma_start(out=sel_sb, in_=sel_d.ap().rearrange("t h p -> h t p"))
            rs_f = sb.tile([HEADS, IC], F32, tag="rs_f")
            nc.vector.reciprocal_approx_fast(out=rs_f, in_=s_sb)
            rs_r = sb.tile([HEADS, IC], F32R, tag="rs_r")
            nc.vector.tensor_copy(out=rs_r, in_=rs_f)
            wo_sb = sb.tile([128, NCT, DIM], BF16, tag="wo")
            nc.sync.dma_start(
                out=wo_sb,
                in_=wo_d.ap().rearrange("(t p) o -> p t o", p=128))
            # normalized bf16 activations: a_n[c, i] = a[c, i] / s[head(c), i]
            a_n = sb.tile([128, NCT, IC], BF16, tag="a_n")
            for ct in range(NCT):
                selr = wl.tile([HEADS, 128], F32R, tag="selr")
                nc.vector.tensor_copy(out=selr, in_=sel_sb[:, ct, :])
                rsb = rb_ps.tile([128, IC], F32, tag="rsb")
                nc.tensor.matmul(rsb, selr, rs_r, start=True, stop=True)
                nc.vector.tensor_mul(a_n[:, ct, :], rsb, a_sb[:, ct, :])
            for m in range(IC // 128):
                osb = ob.tile([128, DIM], BF16, tag="osb")
                for oc in range(2):
                    pp = ps.tile([128, 512], F32, tag="pp")
                    for ct in range(NCT):
                        nc.tensor.matmul(
                            pp, a_n[:, ct, m * 128:(m + 1) * 128],
                            wo_sb[:, ct, oc * 512:(oc + 1) * 512],
                            start=(ct == 0), stop=(ct == NCT - 1))
                    nc.vector.tensor_copy(out=osb[:, oc * 512:(oc + 1) * 512], in_=pp)
                nc.sync.dma_start(out=out_d.ap()[m * 128:(m + 1) * 128, :], in_=osb)

    nc.compile()
    return nc


PROFILE = {"enabled": False, "a_ns": None, "b_ns": None}


def _install_profile_hook():
    """Register the axon NTFF profile hook (the image's antenv lacks
    axon_hooks, so run_bass_kernel_spmd(trace=True) would silently skip
    tracing).  Replicates trn_boot's ctypes recipe."""
    import sys, types, ctypes, contextlib

    if "antenv.axon_hooks" in sys.modules:
        return
    lib = ctypes.CDLL("/opt/axon/libaxon_pjrt.so")
    if not hasattr(lib, "axon_start_nrt_profile"):
        return
    lib.axon_start_nrt_profile.argtypes = [ctypes.POINTER(ctypes.c_int64), ctypes.c_size_t]
    lib.axon_start_nrt_profile.restype = ctypes.c_int64
    lib.axon_stop_nrt_profile.argtypes = [ctypes.c_char_p]
    lib.axon_stop_nrt_profile.restype = ctypes.c_int64

    @contextlib.contextmanager
    def _hook(output_dir, device_ids):
        import jax
        jax.devices()
        if device_ids:
            ids = (ctypes.c_int64 * len(device_ids))(*device_ids)
            rc = lib.axon_start_nrt_profile(ids, len(device_ids))
        else:
            rc = lib.axon_start_nrt_profile(None, 0)
        if rc != 0:
            raise RuntimeError(f"axon_start_nrt_profile rc={rc}")
        try:
            yield
        finally:
            n = lib.axon_stop_nrt_profile(str(output_dir).encode())
            print(f"profile: {n} file(s) written to {output_dir}")

    mod = types.ModuleType("antenv.axon_hooks")
    mod.get_axon_ntff_profile_hook = lambda: _hook
    mod.set_axon_ntff_profile_hook = lambda h: None
    sys.modules["antenv.axon_hooks"] = mod

    # avoid the S3 artifact upload inside the trace path
    from concourse import bass_utils
    bass_utils.upload_artifacts = lambda tmpdir: ""


def kernel(x, gamma, Wq, Wkv, q_scale, k_scale, Wo, rel_pos_bias, mask):
    from concourse.bass_utils import run_bass_kernel_spmd
    import ml_dtypes

    x = np.ascontiguousarray(np.asarray(x, dtype=np.float32))
    gamma = np.asarray(gamma, dtype=np.float32)
    Wq = np.asarray(Wq, dtype=np.float32)
    Wkv = np.asarray(Wkv, dtype=np.float32)
    q_scale = np.asarray(q_scale, dtype=np.float32)
    k_scale = np.asarray(k_scale, dtype=np.float32)
    Wo = np.ascontiguousarray(np.asarray(Wo, dtype=np.float32))
    rel_pos_bias = np.asarray(rel_pos_bias, dtype=np.float32)
    mask = np.asarray(mask)
    mask_ones = bool(mask.all())

    if PROFILE["enabled"]:
        _install_profile_hook()
    akey = ("a", mask_ones)
    if akey not in _cache:
        _cache[akey] = _build_launch_a(mask_ones)
    if "b" not in _cache:
        _cache["b"] = _build_launch_b()

    BF = ml_dtypes.bfloat16
    # host-side prep: transpose/cast x, LN stats, gamma-folded weights,
    # exponentiated transposed positional bias
    xT = np.ascontiguousarray(x.transpose(0, 2, 1)).astype(BF)
    mu = x.mean(-1)
    var = x.var(-1)
    numu = (-mu).astype(BF)                                   # [B, N]
    rstd = (1.0 / np.sqrt(var + LN_EPS)).astype(BF)           # [B, N]
    kb = np.where(mask, 0.0, NEG).astype(np.float32)

    wq_f = gamma[:, None] * Wq
    wk_f = gamma[:, None] * Wkv[:, :DIM]
    wv_f = gamma[:, None] * Wkv[:, DIM:]

    sblk4q = np.zeros((4, 128), np.float32)
    sblk4q[0, 0:64] = q_scale * 8.0
    sblk4q[1, 64:128] = q_scale * 8.0
    sblk4k = np.zeros((4, 128), np.float32)
    sblk4k[2, 0:64] = k_scale
    sblk4k[3, 64:128] = k_scale

    # B = exp(rel_pos_bias^T) in bf16, interleaved [jt, h, p, i] per core
    rpbT = rel_pos_bias.transpose(0, 2, 1)                     # [H, j, i]
    BE = np.exp(rpbT)

    in_maps_a = []
    for c in range(NCORES):
        es = slice(EH * c, EH * (c + 1))
        wq_s = np.ascontiguousarray(wq_f[:, es]).astype(BF)
        wk_s = np.ascontiguousarray(wk_f[:, es]).astype(BF)
        wv_s = np.ascontiguousarray(wv_f[:, es]).astype(BF)
        bint = np.ascontiguousarray(
            BE[2 * c:2 * c + 2].reshape(2, NJT, 128, N).transpose(1, 0, 2, 3)
        ).astype(BF)
        m = {
            "xT": xT,
            "rpbE": bint,
            "wq": wq_s, "wk": wk_s, "wv": wv_s,
            "csq": wq_s.astype(np.float32).sum(0)[None, :].astype(BF),
            "csk": wk_s.astype(np.float32).sum(0)[None, :].astype(BF),
            "csv": wv_s.astype(np.float32).sum(0)[None, :].astype(BF),
            "numu": numu, "rstd": rstd,
            "sblk4q": sblk4q, "sblk4k": sblk4k,
        }
        if not mask_ones:
            m["kb"] = kb
        in_maps_a.append(m)
    res_a = run_bass_kernel_spmd(_cache[akey], in_maps_a, list(range(NCORES)),
                                 trace=PROFILE["enabled"])
    if PROFILE["enabled"]:
        PROFILE["a_ns"] = res_a.exec_time_ns

    AT = np.empty((B, DIM, N), np.float32)
    S = np.empty((B, HEADS, N), np.float32)
    for c in range(NCORES):
        ao = res_a.results[c]["at_out"]            # [B, 2, 65, N]
        for h in range(2):
            AT[:, EH * c + 64 * h:EH * c + 64 * h + 64, :] = ao[:, h, 0:64, :]
            S[:, 2 * c + h, :] = ao[:, h, 64, :]
    AT_bf = AT.astype(BF)
    Wo_bf = Wo.astype(BF)

    sel = np.zeros((NCT, HEADS, 128), np.float32)
    for ct in range(NCT):
        sel[ct, 2 * ct, 0:64] = 1.0
        sel[ct, 2 * ct + 1, 64:128] = 1.0

    in_maps_b = []
    for c in range(NCORES):
        bi, ic = c // NIC, c % NIC
        in_maps_b.append({
            "a_t": np.ascontiguousarray(AT_bf[bi][:, ic * IC:(ic + 1) * IC]),
            "s_slice": np.ascontiguousarray(S[bi][:, ic * IC:(ic + 1) * IC]),
            "sel": sel,
            "wo": Wo_bf,
        })
    res_b = run_bass_kernel_spmd(_cache["b"], in_maps_b, list(range(NCORES)),
                                 trace=PROFILE["enabled"])
    if PROFILE["enabled"]:
        PROFILE["b_ns"] = res_b.exec_time_ns

    out = np.empty((B, N, DIM), np.float32)
    for c in range(NCORES):
        bi, ic = c // NIC, c % NIC
        out[bi, ic * IC:(ic + 1) * IC, :] = res_b.results[c]["out_rows"].astype(np.float32)
    return out


# revision 14
# speedup vs baseline: 1.5895x; 1.2783x over previous
"""Trainium2 Bass kernel for nn_Attention (2-batch, 16-head, n=2048, d=64 causal
attention with LayerNorm-projected l2-normalized q/k, relative position bias,
and output projection), SPMD across 8 NeuronCores.

Sharding: launch A tensor-parallels the 16 heads (2 heads per core, both
batches on every core) and emits transposed attention outputs; launch B
row-shards the final @ Wo matmul across the 8 cores.

Key structure (v2):
 - LayerNorm stats (mean/var) computed on host; gamma folded into the
   projection weights on host; the mean subtraction is a rank-1 matmul
   accumulation; rstd cancels in q/k l2norm and is applied to v.
 - rel_pos_bias enters multiplicatively: host precomputes B = exp(bias^T)
   in bf16, device computes E = exp(sim) straight out of PSUM (one wide
   activation over 4 PSUM banks = 2 j-tiles x 2 heads), then E*B on
   DVE/GpSimd in bf16.  Causal masking = affine_select fill 0.0 on B.
 - sim matmuls for the 2 heads are emitted as adjacent row-tiled (K=64)
   pairs at PE tile positions (0,0)/(64,0) so they can overlap.
 - attn@v uses a 65-wide v||ones stationary; row 64 carries softmax
   denominators; launch B normalizes and row-shards @ Wo in bf16.
 - phase 1 of batch 1 is software-pipelined into phase 2 of batch 0 to
   keep the tensor engine busy during the Act-bound softmax stretches.
"""

import numpy as np

HEADS = 16
DH = 64
B = 2
N = 2048
DIM = 1024
EH = 128          # per-core slice of the inner dim (2 heads x 64)
NCORES = 8
IC = 512          # i-chunk width
NIC = N // IC     # 4 i-chunks
JT = 128          # j-tile width
NJT = N // JT     # 16 j-tiles
NCT = DIM // 128  # 8 contraction tiles
LN_EPS = 1e-5
NEG = -1e30

_cache = {}


def _build_launch_a(mask_ones=True):
    import concourse.bass as bass
    import concourse.tile as tile
    from concourse import bacc, mybir
    from concourse.masks import make_identity

    F32 = mybir.dt.float32
    F32R = mybir.dt.float32r
    BF16 = mybir.dt.bfloat16
    AF = mybir.ActivationFunctionType
    nc = bacc.Bacc(None)
    xT_d = nc.declare_dram_parameter("xT", [B, DIM, N], BF16, isOutput=False)
    rpbE_d = nc.declare_dram_parameter("rpbE", [NJT, 2, 128, N], BF16, isOutput=False)
    wq_d = nc.declare_dram_parameter("wq", [DIM, EH], BF16, isOutput=False)
    wk_d = nc.declare_dram_parameter("wk", [DIM, EH], BF16, isOutput=False)
    wv_d = nc.declare_dram_parameter("wv", [DIM, EH], BF16, isOutput=False)
    csq_d = nc.declare_dram_parameter("csq", [1, EH], BF16, isOutput=False)
    csk_d = nc.declare_dram_parameter("csk", [1, EH], BF16, isOutput=False)
    csv_d = nc.declare_dram_parameter("csv", [1, EH], BF16, isOutput=False)
    numu_d = nc.declare_dram_parameter("numu", [B, N], BF16, isOutput=False)
    rstd_d = nc.declare_dram_parameter("rstd", [B, N], BF16, isOutput=False)
    sbq_d = nc.declare_dram_parameter("sblk4q", [4, 128], F32, isOutput=False)
    sbk_d = nc.declare_dram_parameter("sblk4k", [4, 128], F32, isOutput=False)
    if not mask_ones:
        kb_d = nc.declare_dram_parameter("kb", [B, N], F32, isOutput=False)
    at_d = nc.declare_dram_parameter("at_out", [B, 2, 65, N], F32, isOutput=True)

    with tile.TileContext(nc) as tc:
        import contextlib
        with contextlib.ExitStack() as ctx:
            pers = ctx.enter_context(tc.tile_pool(name="pers", bufs=1))

            # ---------- constants ----------
            onescol_f = pers.tile([128, 1], F32, tag="onescol_f")
            nc.vector.memset(onescol_f, 1.0)
            row_f = pers.tile([1, 128], F32, tag="row_f")
            nc.vector.memset(row_f, 1.0)
            ones_row_bf = pers.tile([1, 128], BF16, tag="ones_row_bf")
            nc.vector.tensor_copy(out=ones_row_bf, in_=row_f)
            ident = pers.tile([128, 128], F32, tag="ident")
            make_identity(nc, ident)
            ident_bf = pers.tile([128, 128], BF16, tag="ident_bf")
            nc.vector.tensor_copy(out=ident_bf, in_=ident)
            eps4 = pers.tile([4, 1], F32, tag="eps4")
            nc.vector.memset(eps4, 1e-24)

            # ssq stationaries: o4q cols 0-1 head-blockdiag, o4k cols 2-3
            o4_f = pers.tile([128, 4], F32, tag="o4_f")
            nc.vector.memset(o4_f, 0.0)
            nc.vector.memset(o4_f[0:64, 0:1], 1.0)
            nc.vector.memset(o4_f[64:128, 1:2], 1.0)
            o4q = pers.tile([128, 4], BF16, tag="o4q")
            nc.vector.tensor_copy(out=o4q, in_=o4_f)
            nc.vector.memset(o4_f, 0.0)
            nc.vector.memset(o4_f[0:64, 2:3], 1.0)
            nc.vector.memset(o4_f[64:128, 3:4], 1.0)
            o4k = pers.tile([128, 4], BF16, tag="o4k")
            nc.vector.tensor_copy(out=o4k, in_=o4_f)

            # scale-broadcast stationaries (f32r)
            sbq_f = pers.tile([4, 128], F32, tag="sbq_f")
            nc.sync.dma_start(out=sbq_f, in_=sbq_d.ap())
            sbq_r = pers.tile([4, 128], F32R, tag="sbq_r")
            nc.vector.tensor_copy(out=sbq_r, in_=sbq_f)
            sbk_f = pers.tile([4, 128], F32, tag="sbk_f")
            nc.sync.dma_start(out=sbk_f, in_=sbk_d.ap())
            sbk_r = pers.tile([4, 128], F32R, tag="sbk_r")
            nc.vector.tensor_copy(out=sbk_r, in_=sbk_f)

            # weights (host gamma-folded), correction rows, LN stats rows
            wps = {}
            css = {}
            for nm, wd, cd in (("q", wq_d, csq_d), ("k", wk_d, csk_d),
                               ("v", wv_d, csv_d)):
                wp = pers.tile([128, NCT, EH], BF16, tag=f"w{nm}p", name=f"wp{nm}")
                nc.sync.dma_start(out=wp, in_=wd.ap().rearrange("(t p) e -> p t e", p=128))
                cs = pers.tile([1, EH], BF16, tag=f"cs{nm}", name=f"cs{nm}")
                nc.sync.dma_start(out=cs, in_=cd.ap())
                wps[nm] = wp
                css[nm] = cs
            numu_sb = pers.tile([1, B, N], BF16, tag="numu_sb")
            nc.sync.dma_start(out=numu_sb, in_=numu_d.ap().unsqueeze(0))
            rstd_sb = pers.tile([1, B, N], BF16, tag="rstd_sb")
            nc.sync.dma_start(out=rstd_sb, in_=rstd_d.ap().unsqueeze(0))
            if not mask_ones:
                kbT = pers.tile([128, B, NJT], F32, tag="kbT")
                nc.sync.dma_start(out=kbT, in_=kb_d.ap().rearrange("b (t p) -> p b t", p=128))

            # persistent per-batch products
            qhat = [pers.tile([128, N], BF16, tag=f"qhat{b}", name=f"qhat{b}") for b in range(B)]
            khat = [pers.tile([128, N], BF16, tag=f"khat{b}", name=f"khat{b}") for b in range(B)]
            v_all = [pers.tile([128, NJT, 130], BF16, tag=f"vall{b}", name=f"vall{b}") for b in range(B)]
            for b in range(B):
                for jt in range(NJT):
                    nc.vector.tensor_copy(out=v_all[b][:, jt, 64:65], in_=onescol_f)
                    nc.vector.tensor_copy(out=v_all[b][:, jt, 129:130], in_=onescol_f)

            # ---------- pools ----------
            sim_ps = ctx.enter_context(tc.tile_pool(name="sim_ps", bufs=1, space="PSUM"))
            av_ps = ctx.enter_context(tc.tile_pool(name="av_ps", bufs=2, space="PSUM"))
            mix_ps = ctx.enter_context(tc.tile_pool(name="mix_ps", bufs=2, space="PSUM"))
            xr_pool = ctx.enter_context(tc.tile_pool(name="xr_pool", bufs=2))
            bc_pool = ctx.enter_context(tc.tile_pool(name="bc_pool", bufs=2))
            bm_pool = ctx.enter_context(tc.tile_pool(name="bm_pool", bufs=2))
            e_pool = ctx.enter_context(tc.tile_pool(name="e_pool", bufs=2))
            m_pool = ctx.enter_context(tc.tile_pool(name="m_pool", bufs=2))
            sq_pool = ctx.enter_context(tc.tile_pool(name="sq_pool", bufs=2))
            rn_pool = ctx.enter_context(tc.tile_pool(name="rn_pool", bufs=1))
            ssq_pool = ctx.enter_context(tc.tile_pool(name="ssq_pool", bufs=1))
            rnr_pool = ctx.enter_context(tc.tile_pool(name="rnr_pool", bufs=1))
            raw_pool = ctx.enter_context(tc.tile_pool(name="raw_pool", bufs=8))
            vsc_pool = ctx.enter_context(tc.tile_pool(name="vsc_pool", bufs=2))
            stg_pool = ctx.enter_context(tc.tile_pool(name="stg_pool", bufs=2))

            # ---------- phase-1 work units ----------
            def ph1_units(b):
                """Emission closures for LN+proj+l2norm of one batch.
                Pass 1 per chunk: projections, squares, ssq; then ONE
                clustered Rsqrt over all chunks (avoids act-table thrash
                with the attention Exp), then per-chunk scale+hat mults."""
                units = []
                state = {}
                ssq_all = ssq_pool.tile([4, NIC, IC], F32, tag="ssqall",
                                        name=f"ssqall{b}")
                rn_r = rnr_pool.tile([4, N], F32R, tag="rnr", name=f"rnr{b}")
                for ic in range(NIC):
                    isl = slice(ic * IC, (ic + 1) * IC)

                    def u_load(b=b, ic=ic, isl=isl):
                        xr = xr_pool.tile([128, NCT, IC], BF16, tag="xr", name="xr")
                        nc.sync.dma_start(
                            out=xr,
                            in_=xT_d.ap()[b, :, isl].rearrange("(t p) n -> p t n", p=128))
                        state[ic] = {"xr": xr}
                    units.append(u_load)

                    def mk_proj(nm, b=b, ic=ic, isl=isl):
                        def u_proj_a():
                            st = state[ic]
                            pp = mix_ps.tile([128, IC], F32, tag="mx", name=f"pp{nm}")
                            for ct in range(4):
                                nc.tensor.matmul(pp, wps[nm][:, ct, :], st["xr"][:, ct, :],
                                                 start=(ct == 0), stop=False)
                            st[f"pp{nm}"] = pp
                        def u_proj_b():
                            st = state[ic]
                            pp = st[f"pp{nm}"]
                            for ct in range(4, NCT):
                                nc.tensor.matmul(pp, wps[nm][:, ct, :], st["xr"][:, ct, :],
                                                 start=False, stop=False)
                            nc.tensor.matmul(pp, css[nm], numu_sb[0:1, b, isl],
                                             start=False, stop=True)
                        return u_proj_a, u_proj_b
                    qa, qb = mk_proj("q")
                    ka, kb_ = mk_proj("k")
                    va, vb = mk_proj("v")

                    def u_qpost(b=b, ic=ic):
                        st = state[ic]
                        q_raw = raw_pool.tile([128, IC], BF16, tag="raw", name="q_raw")
                        nc.vector.tensor_copy(out=q_raw, in_=st["ppq"])
                        sq_q = sq_pool.tile([128, IC], BF16, tag="sq", name="sq_q")
                        nc.vector.tensor_mul(sq_q, q_raw, q_raw)
                        st["q_raw"] = q_raw
                        st["sq_q"] = sq_q

                    def u_kpost(b=b, ic=ic):
                        st = state[ic]
                        k_raw = raw_pool.tile([128, IC], BF16, tag="raw", name="k_raw")
                        nc.vector.tensor_copy(out=k_raw, in_=st["ppk"])
                        sq_k = sq_pool.tile([128, IC], BF16, tag="sq", name="sq_k")
                        nc.vector.tensor_mul(sq_k, k_raw, k_raw)
                        st["k_raw"] = k_raw
                        st["sq_k"] = sq_k

                    def u_ssq(b=b, ic=ic):
                        st = state[ic]
                        ssq4 = mix_ps.tile([4, IC], F32, tag="mx", name="ssq4")
                        nc.tensor.matmul(ssq4, o4q, st["sq_q"], start=True, stop=False)
                        nc.tensor.matmul(ssq4, o4k, st["sq_k"], start=False, stop=True)
                        nc.vector.tensor_copy(out=ssq_all[:, ic, :], in_=ssq4)

                    def u_vpost(b=b, ic=ic, isl=isl):
                        st = state[ic]
                        rstdb = mix_ps.tile([128, IC], F32, tag="mx", name="rstdb")
                        nc.tensor.matmul(rstdb, ones_row_bf, rstd_sb[0:1, b, isl],
                                         start=True, stop=True)
                        rb_sb = vsc_pool.tile([128, IC], F32, tag="rb", name="rb_sb")
                        nc.vector.tensor_copy(out=rb_sb, in_=rstdb)
                        vsc = vsc_pool.tile([128, IC], BF16, tag="vsc", name="vsc")
                        nc.vector.tensor_mul(vsc, st["ppv"], rb_sb)
                        st["vsc"] = vsc

                    def u_vtrans(b=b, ic=ic):
                        st = state[ic]
                        for k in range(IC // 128):
                            jt = ic * (IC // 128) + k
                            vt = mix_ps.tile([128, 128], BF16, tag="mx", name="vt")
                            nc.tensor.transpose(vt, st["vsc"][:, k * 128:(k + 1) * 128], ident_bf)
                            nc.vector.tensor_copy(out=v_all[b][:, jt, 0:64], in_=vt[:, 0:64])
                            nc.vector.tensor_copy(out=v_all[b][:, jt, 65:129], in_=vt[:, 64:128])

                    units += [qa, qb, u_qpost, ka, kb_, u_kpost, u_ssq,
                              va, vb, u_vpost, u_vtrans]

                def u_rsqrt(b=b):
                    rec = rn_pool.tile([4, N], F32, tag="rn", name="rec")
                    nc.vector.reciprocal_approx_fast(out=rec, in_=ssq_all)
                    nc.scalar.activation(out=rn_r, in_=rec, func=AF.Sqrt)
                units.append(u_rsqrt)

                for ic in range(NIC):
                    isl = slice(ic * IC, (ic + 1) * IC)

                    def u_hats(b=b, ic=ic, isl=isl):
                        st = state[ic]
                        sr_q = mix_ps.tile([128, IC], F32, tag="mx", name="sr_q")
                        nc.tensor.matmul(sr_q, sbq_r, rn_r[:, isl], start=True, stop=True)
                        nc.vector.tensor_mul(qhat[b][:, isl], st["q_raw"], sr_q)
                        sr_k = mix_ps.tile([128, IC], F32, tag="mx", name="sr_k")
                        nc.tensor.matmul(sr_k, sbk_r, rn_r[:, isl], start=True, stop=True)
                        nc.vector.tensor_mul(khat[b][:, isl], st["k_raw"], sr_k)
                        del state[ic]
                    units.append(u_hats)
                return units

            # ---------- phase-2 (attention) ----------
            def load_bias_chunk(ic):
                jmax = (IC // 128) * (ic + 1)
                isl = slice(ic * IC, (ic + 1) * IC)
                Bc = bc_pool.tile([128, NJT, 2, IC], BF16, tag="bc", name="Bc")
                nc.sync.dma_start(
                    out=Bc[:, 0:jmax, :, :],
                    in_=rpbE_d.ap()[0:jmax, :, :, isl].rearrange("t h p i -> p t h i"))
                Bm = bm_pool.tile([128, 4, 2, IC], BF16, tag="bm", name="Bm")
                for k in range(4):
                    for h in range(2):
                        nc.gpsimd.affine_select(
                            out=Bm[:, k, h, :], in_=Bc[:, jmax - 4 + k, h, :],
                            compare_op=mybir.AluOpType.is_ge,
                            fill=0.0, base=-128 * k, channel_multiplier=-1,
                            pattern=[[1, IC]])
                return Bc, Bm

            def ph2_chunk(b, ic, Bc, Bm, feed):
                """feed: list of ph1 unit closures to interleave between groups."""
                jmax = (IC // 128) * (ic + 1)
                isl = slice(ic * IC, (ic + 1) * IC)
                avs = [av_ps.tile([65, IC], F32, tag="av", name=f"av{h}")
                       for h in range(2)]
                diag0 = jmax - 4
                for g in range(jmax // 2):
                    jt0 = 2 * g
                    sp = sim_ps.tile([128, 2, 2, IC], F32, tag="sp", name="sp")
                    for l in range(2):
                        jt = jt0 + l
                        # causal trim: diag j-tiles only need i >= jt*128
                        off = max(0, (jt - diag0) * 128) if jt >= diag0 else 0
                        for h in range(2):
                            dsl = slice(64 * h, 64 * h + 64)
                            nc.tensor.matmul(
                                sp[:, l, h, off:],
                                khat[b][dsl, jt * 128:(jt + 1) * 128],
                                qhat[b][dsl, isl.start + off:isl.stop],
                                start=True, stop=True)
                    E = e_pool.tile([128, 2, 2, IC], BF16, tag="E", name="E")
                    if mask_ones:
                        nc.scalar.activation(out=E, in_=sp, func=AF.Exp)
                    else:
                        for l in range(2):
                            jt = jt0 + l
                            for h in range(2):
                                nc.scalar.activation(out=E[:, l, h, :],
                                                     in_=sp[:, l, h, :],
                                                     func=AF.Exp,
                                                     bias=kbT[:, b, jt:jt + 1])
                    Em = m_pool.tile([128, 2, 2, IC], BF16, tag="Em", name="Em")
                    if jt0 >= diag0:
                        bsrc = Bm[:, jt0 - diag0:jt0 - diag0 + 2, :, :]
                    else:
                        bsrc = Bc[:, jt0:jt0 + 2, :, :]
                    nc.vector.tensor_mul(Em, E, bsrc)
                    for l in range(2):
                        jt = jt0 + l
                        off = max(0, (jt - diag0) * 128) if jt >= diag0 else 0
                        for h in range(2):
                            nc.tensor.matmul(
                                avs[h][:, off:], v_all[b][:, jt, 65 * h:65 * h + 65],
                                Em[:, l, h, off:],
                                start=(jt == 0), stop=(jt == jmax - 1))
                    # software-pipeline phase-1 work of the other batch
                    for _ in range(3):
                        if feed:
                            feed.pop(0)()
                for h in range(2):
                    stg = stg_pool.tile([65, IC], F32, tag="stg", name="stg")
                    nc.vector.tensor_copy(out=stg, in_=avs[h][0:65, :])
                    nc.sync.dma_start(out=at_d.ap()[b, h, :, isl], in_=stg)

            # ---------- main schedule ----------
            for u in ph1_units(0):
                u()
            # zero the sim psum banks once so trimmed regions never hold
            # unbounded garbage (exp of it must stay finite)
            sp0 = sim_ps.tile([128, 2, 2, IC], F32, tag="sp", name="sp0")
            nc.vector.memset(sp0, 0.0)
            feed = ph1_units(1)
            # phase 2: all of b0 (absorbing ph1(b1) between groups), then b1
            for b in range(B):
                if b == 1:
                    while feed:
                        feed.pop(0)()
                for ic in range(NIC):
                    Bc, Bm = load_bias_chunk(ic)
                    ph2_chunk(b, ic, Bc, Bm, feed if b == 0 else [])
    nc.compile()
    return nc


def _build_launch_b():
    import concourse.bass as bass
    import concourse.tile as tile
    from concourse import bacc, mybir

    F32 = mybir.dt.float32
    F32R = mybir.dt.float32r
    BF16 = mybir.dt.bfloat16

    nc = bacc.Bacc(None)
    at_d = nc.declare_dram_parameter("a_t", [DIM, IC], BF16, isOutput=False)
    s_d = nc.declare_dram_parameter("s_slice", [HEADS, IC], F32, isOutput=False)
    sel_d = nc.declare_dram_parameter("sel", [NCT, HEADS, 128], F32, isOutput=False)
    wo_d = nc.declare_dram_parameter("wo", [DIM, DIM], BF16, isOutput=False)
    out_d = nc.declare_dram_parameter("out_rows", [IC, DIM], BF16, isOutput=True)

    with tile.TileContext(nc) as tc:
        with tc.tile_pool(name="sb", bufs=1) as sb, \
             tc.tile_pool(name="wl", bufs=2) as wl, \
             tc.tile_pool(name="ob", bufs=2) as ob, \
             tc.tile_pool(name="rb_ps", bufs=2, space="PSUM") as rb_ps, \
             tc.tile_pool(name="ps", bufs=2, space="PSUM") as ps:
            a_sb = sb.tile([128, NCT, IC], BF16, tag="a")
            nc.sync.dma_start(
                out=a_sb,
                in_=at_d.ap().rearrange("(t p) i -> p t i", p=128))
            s_sb = sb.tile([HEADS, IC], F32, tag="s")
            nc.sync.dma_start(out=s_sb, in_=s_d.ap())
            sel_sb = sb.tile([HEADS, NCT, 128], F32, tag="sel")
            nc.sync.dma_start(out=sel_sb, in_=sel_d.ap().rearrange("t h p -> h t p"))
            rs_f = sb.tile([HEADS, IC], F32, tag="rs_f")
            nc.vector.reciprocal_approx_fast(out=rs_f, in_=s_sb)
            rs_r = sb.tile([HEADS, IC], F32R, tag="rs_r")
            nc.vector.tensor_copy(out=rs_r, in_=rs_f)
            wo_sb = sb.tile([128, NCT, DIM], BF16, tag="wo")
            nc.sync.dma_start(
                out=wo_sb,
                in_=wo_d.ap().rearrange("(t p) o -> p t o", p=128))
            # normalized bf16 activations: a_n[c, i] = a[c, i] / s[head(c), i]
            a_n = sb.tile([128, NCT, IC], BF16, tag="a_n")
            for ct in range(NCT):
                selr = wl.tile([HEADS, 128], F32R, tag="selr")
                nc.vector.tensor_copy(out=selr, in_=sel_sb[:, ct, :])
                rsb = rb_ps.tile([128, IC], F32, tag="rsb")
                nc.tensor.matmul(rsb, selr, rs_r, start=True, stop=True)
                nc.vector.tensor_mul(a_n[:, ct, :], rsb, a_sb[:, ct, :])
            for m in range(IC // 128):
                osb = ob.tile([128, DIM], BF16, tag="osb")
                for oc in range(2):
                    pp = ps.tile([128, 512], F32, tag="pp")
                    for ct in range(NCT):
                        nc.tensor.matmul(
                            pp, a_n[:, ct, m * 128:(m + 1) * 128],
                            wo_sb[:, ct, oc * 512:(oc + 1) * 512],
                            start=(ct == 0), stop=(ct == NCT - 1))
                    nc.vector.tensor_copy(out=osb[:, oc * 512:(oc + 1) * 512], in_=pp)
                nc.sync.dma_start(out=out_d.ap()[m * 128:(m + 1) * 128, :], in_=osb)

    nc.compile()
    return nc


PROFILE = {"enabled": False, "a_ns": None, "b_ns": None}


def _install_profile_hook():
    """Register the axon NTFF profile hook (the image's antenv lacks
    axon_hooks, so run_bass_kernel_spmd(trace=True) would silently skip
    tracing).  Replicates trn_boot's ctypes recipe."""
    import sys, types, ctypes, contextlib

    if "antenv.axon_hooks" in sys.modules:
        return
    lib = ctypes.CDLL("/opt/axon/libaxon_pjrt.so")
    if not hasattr(lib, "axon_start_nrt_profile"):
        return
    lib.axon_start_nrt_profile.argtypes = [ctypes.POINTER(ctypes.c_int64), ctypes.c_size_t]
    lib.axon_start_nrt_profile.restype = ctypes.c_int64
    lib.axon_stop_nrt_profile.argtypes = [ctypes.c_char_p]
    lib.axon_stop_nrt_profile.restype = ctypes.c_int64

    @contextlib.contextmanager
    def _hook(output_dir, device_ids):
        import jax
        jax.devices()
        if device_ids:
            ids = (ctypes.c_int64 * len(device_ids))(*device_ids)
            rc = lib.axon_start_nrt_profile(ids, len(device_ids))
        else:
            rc = lib.axon_start_nrt_profile(None, 0)
        if rc != 0:
            raise RuntimeError(f"axon_start_nrt_profile rc={rc}")
        try:
            yield
        finally:
            n = lib.axon_stop_nrt_profile(str(output_dir).encode())
            print(f"profile: {n} file(s) written to {output_dir}")

    mod = types.ModuleType("antenv.axon_hooks")
    mod.get_axon_ntff_profile_hook = lambda: _hook
    mod.set_axon_ntff_profile_hook = lambda h: None
    sys.modules["antenv.axon_hooks"] = mod

    # avoid the S3 artifact upload inside the trace path
    from concourse import bass_utils
    bass_utils.upload_artifacts = lambda tmpdir: ""


def kernel(x, gamma, Wq, Wkv, q_scale, k_scale, Wo, rel_pos_bias, mask):
    from concourse.bass_utils import run_bass_kernel_spmd
    import ml_dtypes

    x = np.ascontiguousarray(np.asarray(x, dtype=np.float32))
    gamma = np.asarray(gamma, dtype=np.float32)
    Wq = np.asarray(Wq, dtype=np.float32)
    Wkv = np.asarray(Wkv, dtype=np.float32)
    q_scale = np.asarray(q_scale, dtype=np.float32)
    k_scale = np.asarray(k_scale, dtype=np.float32)
    Wo = np.ascontiguousarray(np.asarray(Wo, dtype=np.float32))
    rel_pos_bias = np.asarray(rel_pos_bias, dtype=np.float32)
    mask = np.asarray(mask)
    mask_ones = bool(mask.all())

    if PROFILE["enabled"]:
        _install_profile_hook()
    akey = ("a", mask_ones)
    if akey not in _cache:
        _cache[akey] = _build_launch_a(mask_ones)
    if "b" not in _cache:
        _cache["b"] = _build_launch_b()

    BF = ml_dtypes.bfloat16
    # host-side prep: transpose/cast x, LN stats, gamma-folded weights,
    # exponentiated transposed positional bias
    xT = np.ascontiguousarray(x.transpose(0, 2, 1)).astype(BF)
    mu = x.mean(-1)
    var = x.var(-1)
    numu = (-mu).astype(BF)                                   # [B, N]
    rstd = (1.0 / np.sqrt(var + LN_EPS)).astype(BF)           # [B, N]
    kb = np.where(mask, 0.0, NEG).astype(np.float32)

    wq_f = gamma[:, None] * Wq
    wk_f = gamma[:, None] * Wkv[:, :DIM]
    wv_f = gamma[:, None] * Wkv[:, DIM:]

    sblk4q = np.zeros((4, 128), np.float32)
    sblk4q[0, 0:64] = q_scale * 8.0
    sblk4q[1, 64:128] = q_scale * 8.0
    sblk4k = np.zeros((4, 128), np.float32)
    sblk4k[2, 0:64] = k_scale
    sblk4k[3, 64:128] = k_scale

    # B = exp(rel_pos_bias^T) in bf16, interleaved [jt, h, p, i] per core
    rpbT = rel_pos_bias.transpose(0, 2, 1)                     # [H, j, i]
    BE = np.exp(rpbT)

    in_maps_a = []
    for c in range(NCORES):
        es = slice(EH * c, EH * (c + 1))
        wq_s = np.ascontiguousarray(wq_f[:, es]).astype(BF)
        wk_s = np.ascontiguousarray(wk_f[:, es]).astype(BF)
        wv_s = np.ascontiguousarray(wv_f[:, es]).astype(BF)
        bint = np.ascontiguousarray(
            BE[2 * c:2 * c + 2].reshape(2, NJT, 128, N).transpose(1, 0, 2, 3)
        ).astype(BF)
        m = {
            "xT": xT,
            "rpbE": bint,
            "wq": wq_s, "wk": wk_s, "wv": wv_s,
            "csq": wq_s.astype(np.float32).sum(0)[None, :].astype(BF),
            "csk": wk_s.astype(np.float32).sum(0)[None, :].astype(BF),
            "csv": wv_s.astype(np.float32).sum(0)[None, :].astype(BF),
            "numu": numu, "rstd": rstd,
            "sblk4q": sblk4q, "sblk4k": sblk4k,
        }
        if not mask_ones:
            m["kb"] = kb
        in_maps_a.append(m)
    res_a = run_bass_kernel_spmd(_cache[akey], in_maps_a, list(range(NCORES)),
                                 trace=PROFILE["enabled"])
    if PROFILE["enabled"]:
        PROFILE["a_ns"] = res_a.exec_time_ns

    AT = np.empty((B, DIM, N), np.float32)
    S = np.empty((B, HEADS, N), np.float32)
    for c in range(NCORES):
        ao = res_a.results[c]["at_out"]            # [B, 2, 65, N]
        for h in range(2):
            AT[:, EH * c + 64 * h:EH * c + 64 * h + 64, :] = ao[:, h, 0:64, :]
            S[:, 2 * c + h, :] = ao[:, h, 64, :]
    AT_bf = AT.astype(BF)
    Wo_bf = Wo.astype(BF)

    sel = np.zeros((NCT, HEADS, 128), np.float32)
    for ct in range(NCT):
        sel[ct, 2 * ct, 0:64] = 1.0
        sel[ct, 2 * ct + 1, 64:128] = 1.0

    in_maps_b = []
    for c in range(NCORES):
        bi, ic = c // NIC, c % NIC
        in_maps_b.append({
            "a_t": np.ascontiguousarray(AT_bf[bi][:, ic * IC:(ic + 1) * IC]),
            "s_slice": np.ascontiguousarray(S[bi][:, ic * IC:(ic + 1) * IC]),
            "sel": sel,
            "wo": Wo_bf,
        })
    res_b = run_bass_kernel_spmd(_cache["b"], in_maps_b, list(range(NCORES)),
                                 trace=PROFILE["enabled"])
    if PROFILE["enabled"]:
        PROFILE["b_ns"] = res_b.exec_time_ns

    out = np.empty((B, N, DIM), np.float32)
    for c in range(NCORES):
        bi, ic = c // NIC, c % NIC
        out[bi, ic * IC:(ic + 1) * IC, :] = res_b.results[c]["out_rows"].astype(np.float32)
    return out


# revision 18
# speedup vs baseline: 1.6210x; 1.0198x over previous
"""Trainium2 Bass kernel for nn_Attention (2-batch, 16-head, n=2048, d=64 causal
attention with LayerNorm-projected l2-normalized q/k, relative position bias,
and output projection), SPMD across 8 NeuronCores.

Sharding: launch A tensor-parallels the 16 heads (2 heads per core, both
batches on every core) and emits transposed attention outputs; launch B
row-shards the final @ Wo matmul across the 8 cores.

Key structure (v2):
 - LayerNorm stats (mean/var) computed on host; gamma folded into the
   projection weights on host; the mean subtraction is a rank-1 matmul
   accumulation; rstd cancels in q/k l2norm and is applied to v.
 - rel_pos_bias enters multiplicatively: host precomputes B = exp(bias^T)
   in bf16, device computes E = exp(sim) straight out of PSUM (one wide
   activation over 4 PSUM banks = 2 j-tiles x 2 heads), then E*B on
   DVE/GpSimd in bf16.  Causal masking = affine_select fill 0.0 on B.
 - sim matmuls for the 2 heads are emitted as adjacent row-tiled (K=64)
   pairs at PE tile positions (0,0)/(64,0) so they can overlap.
 - attn@v uses a 65-wide v||ones stationary; row 64 carries softmax
   denominators; launch B normalizes and row-shards @ Wo in bf16.
 - phase 1 of batch 1 is software-pipelined into phase 2 of batch 0 to
   keep the tensor engine busy during the Act-bound softmax stretches.
"""

import numpy as np

HEADS = 16
DH = 64
B = 2
N = 2048
DIM = 1024
EH = 128          # per-core slice of the inner dim (2 heads x 64)
NCORES = 8
IC = 512          # i-chunk width
NIC = N // IC     # 4 i-chunks
JT = 128          # j-tile width
NJT = N // JT     # 16 j-tiles
NCT = DIM // 128  # 8 contraction tiles
LN_EPS = 1e-5
NEG = -1e30

_cache = {}


def _build_launch_a(mask_ones=True):
    import concourse.bass as bass
    import concourse.tile as tile
    from concourse import bacc, mybir
    from concourse.masks import make_identity

    F32 = mybir.dt.float32
    F32R = mybir.dt.float32r
    BF16 = mybir.dt.bfloat16
    AF = mybir.ActivationFunctionType
    nc = bacc.Bacc(None)
    xT_d = nc.declare_dram_parameter("xT", [B, DIM, N], BF16, isOutput=False)
    rpbE_d = nc.declare_dram_parameter("rpbE", [NJT, 2, 128, N], BF16, isOutput=False)
    wq_d = nc.declare_dram_parameter("wq", [DIM, EH], BF16, isOutput=False)
    wk_d = nc.declare_dram_parameter("wk", [DIM, EH], BF16, isOutput=False)
    wv_d = nc.declare_dram_parameter("wv", [DIM, EH], BF16, isOutput=False)
    csq_d = nc.declare_dram_parameter("csq", [1, EH], BF16, isOutput=False)
    csk_d = nc.declare_dram_parameter("csk", [1, EH], BF16, isOutput=False)
    csv_d = nc.declare_dram_parameter("csv", [1, EH], BF16, isOutput=False)
    numu_d = nc.declare_dram_parameter("numu", [B, N], BF16, isOutput=False)
    rstd_d = nc.declare_dram_parameter("rstd", [B, N], BF16, isOutput=False)
    sbq_d = nc.declare_dram_parameter("sblk4q", [4, 128], F32, isOutput=False)
    sbk_d = nc.declare_dram_parameter("sblk4k", [4, 128], F32, isOutput=False)
    if not mask_ones:
        kb_d = nc.declare_dram_parameter("kb", [B, N], F32, isOutput=False)
    at_d = nc.declare_dram_parameter("at_out", [B, 2, 65, N], F32, isOutput=True)

    with tile.TileContext(nc) as tc:
        import contextlib
        with contextlib.ExitStack() as ctx:
            pers = ctx.enter_context(tc.tile_pool(name="pers", bufs=1))

            # ---------- constants ----------
            onescol_f = pers.tile([128, 1], F32, tag="onescol_f")
            nc.vector.memset(onescol_f, 1.0)
            row_f = pers.tile([1, 128], F32, tag="row_f")
            nc.vector.memset(row_f, 1.0)
            ones_row_bf = pers.tile([1, 128], BF16, tag="ones_row_bf")
            nc.vector.tensor_copy(out=ones_row_bf, in_=row_f)
            ident = pers.tile([128, 128], F32, tag="ident")
            make_identity(nc, ident)
            ident_bf = pers.tile([128, 128], BF16, tag="ident_bf")
            nc.vector.tensor_copy(out=ident_bf, in_=ident)
            eps4 = pers.tile([4, 1], F32, tag="eps4")
            nc.vector.memset(eps4, 1e-24)

            # ssq stationaries: o4q cols 0-1 head-blockdiag, o4k cols 2-3
            o4_f = pers.tile([128, 4], F32, tag="o4_f")
            nc.vector.memset(o4_f, 0.0)
            nc.vector.memset(o4_f[0:64, 0:1], 1.0)
            nc.vector.memset(o4_f[64:128, 1:2], 1.0)
            o4q = pers.tile([128, 4], BF16, tag="o4q")
            nc.vector.tensor_copy(out=o4q, in_=o4_f)
            nc.vector.memset(o4_f, 0.0)
            nc.vector.memset(o4_f[0:64, 2:3], 1.0)
            nc.vector.memset(o4_f[64:128, 3:4], 1.0)
            o4k = pers.tile([128, 4], BF16, tag="o4k")
            nc.vector.tensor_copy(out=o4k, in_=o4_f)

            # scale-broadcast stationaries (f32r)
            sbq_f = pers.tile([4, 128], F32, tag="sbq_f")
            nc.sync.dma_start(out=sbq_f, in_=sbq_d.ap())
            sbq_r = pers.tile([4, 128], F32R, tag="sbq_r")
            nc.vector.tensor_copy(out=sbq_r, in_=sbq_f)
            sbk_f = pers.tile([4, 128], F32, tag="sbk_f")
            nc.sync.dma_start(out=sbk_f, in_=sbk_d.ap())
            sbk_r = pers.tile([4, 128], F32R, tag="sbk_r")
            nc.vector.tensor_copy(out=sbk_r, in_=sbk_f)

            # weights (host gamma-folded), correction rows, LN stats rows
            wps = {}
            css = {}
            for nm, wd, cd in (("q", wq_d, csq_d), ("k", wk_d, csk_d),
                               ("v", wv_d, csv_d)):
                wp = pers.tile([128, NCT, EH], BF16, tag=f"w{nm}p", name=f"wp{nm}")
                nc.sync.dma_start(out=wp, in_=wd.ap().rearrange("(t p) e -> p t e", p=128))
                cs = pers.tile([1, EH], BF16, tag=f"cs{nm}", name=f"cs{nm}")
                nc.sync.dma_start(out=cs, in_=cd.ap())
                wps[nm] = wp
                css[nm] = cs
            numu_sb = pers.tile([1, B, N], BF16, tag="numu_sb")
            nc.sync.dma_start(out=numu_sb, in_=numu_d.ap().unsqueeze(0))
            rstd_sb = pers.tile([1, B, N], BF16, tag="rstd_sb")
            nc.sync.dma_start(out=rstd_sb, in_=rstd_d.ap().unsqueeze(0))
            if not mask_ones:
                kbT = pers.tile([128, B, NJT], F32, tag="kbT")
                nc.sync.dma_start(out=kbT, in_=kb_d.ap().rearrange("b (t p) -> p b t", p=128))

            # persistent per-batch products
            qhat = [pers.tile([128, N], BF16, tag=f"qhat{b}", name=f"qhat{b}") for b in range(B)]
            khat = [pers.tile([128, N], BF16, tag=f"khat{b}", name=f"khat{b}") for b in range(B)]
            v_all = [pers.tile([128, NJT, 130], BF16, tag=f"vall{b}", name=f"vall{b}") for b in range(B)]
            for b in range(B):
                for jt in range(NJT):
                    nc.vector.tensor_copy(out=v_all[b][:, jt, 64:65], in_=onescol_f)
                    nc.vector.tensor_copy(out=v_all[b][:, jt, 129:130], in_=onescol_f)

            # ---------- pools ----------
            sim_ps = ctx.enter_context(tc.tile_pool(name="sim_ps", bufs=2, space="PSUM"))
            av_ps = ctx.enter_context(tc.tile_pool(name="av_ps", bufs=2, space="PSUM"))
            mix_ps = ctx.enter_context(tc.tile_pool(name="mix_ps", bufs=2, space="PSUM"))
            xr_pool = ctx.enter_context(tc.tile_pool(name="xr_pool", bufs=2))
            bc_pool = ctx.enter_context(tc.tile_pool(name="bc_pool", bufs=2))
            e_pool = ctx.enter_context(tc.tile_pool(name="e_pool", bufs=2))
            m_pool = ctx.enter_context(tc.tile_pool(name="m_pool", bufs=2))
            sq_pool = ctx.enter_context(tc.tile_pool(name="sq_pool", bufs=2))
            rn_pool = ctx.enter_context(tc.tile_pool(name="rn_pool", bufs=1))
            ssq_pool = ctx.enter_context(tc.tile_pool(name="ssq_pool", bufs=1))
            rnr_pool = ctx.enter_context(tc.tile_pool(name="rnr_pool", bufs=1))
            raw_pool = ctx.enter_context(tc.tile_pool(name="raw_pool", bufs=8))
            vsc_pool = ctx.enter_context(tc.tile_pool(name="vsc_pool", bufs=2))
            stg_pool = ctx.enter_context(tc.tile_pool(name="stg_pool", bufs=2))

            # ---------- phase-1 work units ----------
            def ph1_units(b):
                """Emission closures for LN+proj+l2norm of one batch.
                Pass 1 per chunk: projections, squares, ssq; then ONE
                clustered Rsqrt over all chunks (avoids act-table thrash
                with the attention Exp), then per-chunk scale+hat mults."""
                units = []
                state = {}
                ssq_all = ssq_pool.tile([4, NIC, IC], F32, tag="ssqall",
                                        name=f"ssqall{b}")
                rn_r = rnr_pool.tile([4, N], F32R, tag="rnr", name=f"rnr{b}")
                for ic in range(NIC):
                    isl = slice(ic * IC, (ic + 1) * IC)

                    def u_load(b=b, ic=ic, isl=isl):
                        xr = xr_pool.tile([128, NCT, IC], BF16, tag="xr", name="xr")
                        nc.sync.dma_start(
                            out=xr,
                            in_=xT_d.ap()[b, :, isl].rearrange("(t p) n -> p t n", p=128))
                        state[ic] = {"xr": xr}
                    units.append(u_load)

                    def mk_proj(nm, b=b, ic=ic, isl=isl):
                        def u_proj_a():
                            st = state[ic]
                            pp = mix_ps.tile([128, IC], F32, tag="mx", name=f"pp{nm}")
                            for ct in range(4):
                                nc.tensor.matmul(pp, wps[nm][:, ct, :], st["xr"][:, ct, :],
                                                 start=(ct == 0), stop=False)
                            st[f"pp{nm}"] = pp
                        def u_proj_b():
                            st = state[ic]
                            pp = st[f"pp{nm}"]
                            for ct in range(4, NCT):
                                nc.tensor.matmul(pp, wps[nm][:, ct, :], st["xr"][:, ct, :],
                                                 start=False, stop=False)
                            nc.tensor.matmul(pp, css[nm], numu_sb[0:1, b, isl],
                                             start=False, stop=True)
                        return u_proj_a, u_proj_b
                    qa, qb = mk_proj("q")
                    ka, kb_ = mk_proj("k")
                    va, vb = mk_proj("v")

                    def u_qpost(b=b, ic=ic):
                        st = state[ic]
                        q_raw = raw_pool.tile([128, IC], BF16, tag="raw", name="q_raw")
                        nc.vector.tensor_copy(out=q_raw, in_=st["ppq"])
                        sq_q = sq_pool.tile([128, IC], BF16, tag="sq", name="sq_q")
                        nc.vector.tensor_mul(sq_q, q_raw, q_raw)
                        st["q_raw"] = q_raw
                        st["sq_q"] = sq_q

                    def u_kpost(b=b, ic=ic):
                        st = state[ic]
                        k_raw = raw_pool.tile([128, IC], BF16, tag="raw", name="k_raw")
                        nc.vector.tensor_copy(out=k_raw, in_=st["ppk"])
                        sq_k = sq_pool.tile([128, IC], BF16, tag="sq", name="sq_k")
                        nc.vector.tensor_mul(sq_k, k_raw, k_raw)
                        st["k_raw"] = k_raw
                        st["sq_k"] = sq_k

                    def u_ssq(b=b, ic=ic):
                        st = state[ic]
                        ssq4 = mix_ps.tile([4, IC], F32, tag="mx", name="ssq4")
                        nc.tensor.matmul(ssq4, o4q, st["sq_q"], start=True, stop=False)
                        nc.tensor.matmul(ssq4, o4k, st["sq_k"], start=False, stop=True)
                        nc.vector.tensor_copy(out=ssq_all[:, ic, :], in_=ssq4)

                    def u_vpost(b=b, ic=ic, isl=isl):
                        st = state[ic]
                        rstdb = mix_ps.tile([128, IC], F32, tag="mx", name="rstdb")
                        nc.tensor.matmul(rstdb, ones_row_bf, rstd_sb[0:1, b, isl],
                                         start=True, stop=True)
                        rb_sb = vsc_pool.tile([128, IC], F32, tag="rb", name="rb_sb")
                        nc.vector.tensor_copy(out=rb_sb, in_=rstdb)
                        vsc = vsc_pool.tile([128, IC], BF16, tag="vsc", name="vsc")
                        nc.vector.tensor_mul(vsc, st["ppv"], rb_sb)
                        st["vsc"] = vsc

                    def u_vtrans(b=b, ic=ic):
                        st = state[ic]
                        for k in range(IC // 128):
                            jt = ic * (IC // 128) + k
                            vt = mix_ps.tile([128, 128], BF16, tag="mx", name="vt")
                            nc.tensor.transpose(vt, st["vsc"][:, k * 128:(k + 1) * 128], ident_bf)
                            nc.vector.tensor_copy(out=v_all[b][:, jt, 0:64], in_=vt[:, 0:64])
                            nc.vector.tensor_copy(out=v_all[b][:, jt, 65:129], in_=vt[:, 64:128])

                    units += [qa, qb, u_qpost, ka, kb_, u_kpost, u_ssq,
                              va, vb, u_vpost, u_vtrans]

                def u_rsqrt(b=b):
                    rec = rn_pool.tile([4, N], F32, tag="rn", name="rec")
                    nc.vector.reciprocal_approx_fast(out=rec, in_=ssq_all)
                    nc.scalar.activation(out=rn_r, in_=rec, func=AF.Sqrt)
                units.append(u_rsqrt)

                for ic in range(NIC):
                    isl = slice(ic * IC, (ic + 1) * IC)

                    def u_hats(b=b, ic=ic, isl=isl):
                        st = state[ic]
                        sr_q = mix_ps.tile([128, IC], F32, tag="mx", name="sr_q")
                        nc.tensor.matmul(sr_q, sbq_r, rn_r[:, isl], start=True, stop=True)
                        nc.vector.tensor_mul(qhat[b][:, isl], st["q_raw"], sr_q)
                        sr_k = mix_ps.tile([128, IC], F32, tag="mx", name="sr_k")
                        nc.tensor.matmul(sr_k, sbk_r, rn_r[:, isl], start=True, stop=True)
                        nc.vector.tensor_mul(khat[b][:, isl], st["k_raw"], sr_k)
                        del state[ic]
                    units.append(u_hats)
                return units

            # ---------- phase-2 (attention) ----------
            def load_bias_chunk(ic):
                jmax = (IC // 128) * (ic + 1)
                isl = slice(ic * IC, (ic + 1) * IC)
                Bc = bc_pool.tile([128, NJT, 2, IC], BF16, tag="bc", name="Bc")
                nc.sync.dma_start(
                    out=Bc[:, 0:jmax, :, :],
                    in_=rpbE_d.ap()[0:jmax, :, :, isl].rearrange("t h p i -> p t h i"))
                # causal mask: zero B above the diagonal, in place, trimmed to
                # the valid suffix (the masked prefix is never read by attn@v)
                for k in range(4):
                    w = IC - 128 * k
                    for h in range(2):
                        nc.gpsimd.affine_select(
                            out=Bc[:, jmax - 4 + k, h, 128 * k:],
                            in_=Bc[:, jmax - 4 + k, h, 128 * k:],
                            compare_op=mybir.AluOpType.is_ge,
                            fill=0.0, base=0, channel_multiplier=-1,
                            pattern=[[1, w]])
                return Bc

            def ph2_chunk(b, ic, Bc, feed):
                """feed: list of ph1 unit closures to interleave between groups."""
                jmax = (IC // 128) * (ic + 1)
                isl = slice(ic * IC, (ic + 1) * IC)
                avs = [av_ps.tile([65, IC], F32, tag="av", name=f"av{h}")
                       for h in range(2)]
                diag0 = jmax - 4
                for jt in range(jmax):
                    # causal trim: diag j-tiles only need i >= jt*128
                    off = max(0, (jt - diag0) * 128)
                    sp = sim_ps.tile([128, 2, IC], F32, tag="sp", name="sp")
                    for h in range(2):
                        dsl = slice(64 * h, 64 * h + 64)
                        nc.tensor.matmul(
                            sp[:, h, off:],
                            khat[b][dsl, jt * 128:(jt + 1) * 128],
                            qhat[b][dsl, isl.start + off:isl.stop],
                            start=True, stop=True)
                    E = e_pool.tile([128, 2, IC], BF16, tag="E", name="E")
                    if mask_ones:
                        nc.scalar.activation(out=E, in_=sp, func=AF.Exp)
                    else:
                        for h in range(2):
                            nc.scalar.activation(out=E[:, h, :],
                                                 in_=sp[:, h, :],
                                                 func=AF.Exp,
                                                 bias=kbT[:, b, jt:jt + 1])
                    Em = m_pool.tile([128, 2, IC], BF16, tag="Em", name="Em")
                    nc.vector.tensor_mul(Em, E, Bc[:, jt, :, :])
                    for h in range(2):
                        nc.tensor.matmul(
                            avs[h][:, off:], v_all[b][:, jt, 65 * h:65 * h + 65],
                            Em[:, h, off:],
                            start=(jt == 0), stop=(jt == jmax - 1))
                    # software-pipeline phase-1 work of the other batch
                    if feed:
                        feed.pop(0)()
                for h in range(2):
                    stg = stg_pool.tile([65, IC], F32, tag="stg", name="stg")
                    nc.vector.tensor_copy(out=stg, in_=avs[h][0:65, :])
                    nc.sync.dma_start(out=at_d.ap()[b, h, :, isl], in_=stg)

            # ---------- main schedule ----------
            for u in ph1_units(0):
                u()
            # zero the sim psum banks once so trimmed regions never hold
            # unbounded garbage (exp of it must stay finite)
            for i in range(2):
                sp0 = sim_ps.tile([128, 2, IC], F32, tag="sp", name="sp0")
                nc.vector.memset(sp0, 0.0)
            feed = ph1_units(1)
            # phase 2: all of b0 (absorbing ph1(b1) between groups), then b1
            for b in range(B):
                if b == 1:
                    while feed:
                        feed.pop(0)()
                for ic in range(NIC):
                    Bc = load_bias_chunk(ic)
                    ph2_chunk(b, ic, Bc, feed if b == 0 else [])
    nc.compile()
    return nc


def _build_launch_b():
    import concourse.bass as bass
    import concourse.tile as tile
    from concourse import bacc, mybir

    F32 = mybir.dt.float32
    F32R = mybir.dt.float32r
    BF16 = mybir.dt.bfloat16

    nc = bacc.Bacc(None)
    at_d = nc.declare_dram_parameter("a_t", [DIM, IC], BF16, isOutput=False)
    s_d = nc.declare_dram_parameter("s_slice", [HEADS, IC], F32, isOutput=False)
    sel_d = nc.declare_dram_parameter("sel", [NCT, HEADS, 128], F32, isOutput=False)
    wo_d = nc.declare_dram_parameter("wo", [DIM, DIM], BF16, isOutput=False)
    out_d = nc.declare_dram_parameter("out_rows", [IC, DIM], BF16, isOutput=True)

    with tile.TileContext(nc) as tc:
        with tc.tile_pool(name="sb", bufs=1) as sb, \
             tc.tile_pool(name="wl", bufs=2) as wl, \
             tc.tile_pool(name="ob", bufs=2) as ob, \
             tc.tile_pool(name="rb_ps", bufs=2, space="PSUM") as rb_ps, \
             tc.tile_pool(name="ps", bufs=2, space="PSUM") as ps:
            a_sb = sb.tile([128, NCT, IC], BF16, tag="a")
            nc.sync.dma_start(
                out=a_sb,
                in_=at_d.ap().rearrange("(t p) i -> p t i", p=128))
            s_sb = sb.tile([HEADS, IC], F32, tag="s")
            nc.sync.dma_start(out=s_sb, in_=s_d.ap())
            sel_sb = sb.tile([HEADS, NCT, 128], F32, tag="sel")
            nc.sync.dma_start(out=sel_sb, in_=sel_d.ap().rearrange("t h p -> h t p"))
            rs_f = sb.tile([HEADS, IC], F32, tag="rs_f")
            nc.vector.reciprocal_approx_fast(out=rs_f, in_=s_sb)
            rs_r = sb.tile([HEADS, IC], F32R, tag="rs_r")
            nc.vector.tensor_copy(out=rs_r, in_=rs_f)
            wo_sb = sb.tile([128, NCT, DIM], BF16, tag="wo")
            nc.sync.dma_start(
                out=wo_sb,
                in_=wo_d.ap().rearrange("(t p) o -> p t o", p=128))
            # normalized bf16 activations: a_n[c, i] = a[c, i] / s[head(c), i]
            a_n = sb.tile([128, NCT, IC], BF16, tag="a_n")
            for ct in range(NCT):
                selr = wl.tile([HEADS, 128], F32R, tag="selr")
                nc.vector.tensor_copy(out=selr, in_=sel_sb[:, ct, :])
                rsb = rb_ps.tile([128, IC], F32, tag="rsb")
                nc.tensor.matmul(rsb, selr, rs_r, start=True, stop=True)
                nc.vector.tensor_mul(a_n[:, ct, :], rsb, a_sb[:, ct, :])
            for m in range(IC // 128):
                osb = ob.tile([128, DIM], BF16, tag="osb")
                for oc in range(2):
                    pp = ps.tile([128, 512], F32, tag="pp")
                    for ct in range(NCT):
                        nc.tensor.matmul(
                            pp, a_n[:, ct, m * 128:(m + 1) * 128],
                            wo_sb[:, ct, oc * 512:(oc + 1) * 512],
                            start=(ct == 0), stop=(ct == NCT - 1))
                    nc.vector.tensor_copy(out=osb[:, oc * 512:(oc + 1) * 512], in_=pp)
                nc.sync.dma_start(out=out_d.ap()[m * 128:(m + 1) * 128, :], in_=osb)

    nc.compile()
    return nc


PROFILE = {"enabled": False, "a_ns": None, "b_ns": None}


def _install_profile_hook():
    """Register the axon NTFF profile hook (the image's antenv lacks
    axon_hooks, so run_bass_kernel_spmd(trace=True) would silently skip
    tracing).  Replicates trn_boot's ctypes recipe."""
    import sys, types, ctypes, contextlib

    if "antenv.axon_hooks" in sys.modules:
        return
    lib = ctypes.CDLL("/opt/axon/libaxon_pjrt.so")
    if not hasattr(lib, "axon_start_nrt_profile"):
        return
    lib.axon_start_nrt_profile.argtypes = [ctypes.POINTER(ctypes.c_int64), ctypes.c_size_t]
    lib.axon_start_nrt_profile.restype = ctypes.c_int64
    lib.axon_stop_nrt_profile.argtypes = [ctypes.c_char_p]
    lib.axon_stop_nrt_profile.restype = ctypes.c_int64

    @contextlib.contextmanager
    def _hook(output_dir, device_ids):
        import jax
        jax.devices()
        if device_ids:
            ids = (ctypes.c_int64 * len(device_ids))(*device_ids)
            rc = lib.axon_start_nrt_profile(ids, len(device_ids))
        else:
            rc = lib.axon_start_nrt_profile(None, 0)
        if rc != 0:
            raise RuntimeError(f"axon_start_nrt_profile rc={rc}")
        try:
            yield
        finally:
            n = lib.axon_stop_nrt_profile(str(output_dir).encode())
            print(f"profile: {n} file(s) written to {output_dir}")

    mod = types.ModuleType("antenv.axon_hooks")
    mod.get_axon_ntff_profile_hook = lambda: _hook
    mod.set_axon_ntff_profile_hook = lambda h: None
    sys.modules["antenv.axon_hooks"] = mod

    # avoid the S3 artifact upload inside the trace path
    from concourse import bass_utils
    bass_utils.upload_artifacts = lambda tmpdir: ""


def kernel(x, gamma, Wq, Wkv, q_scale, k_scale, Wo, rel_pos_bias, mask):
    from concourse.bass_utils import run_bass_kernel_spmd
    import ml_dtypes

    x = np.ascontiguousarray(np.asarray(x, dtype=np.float32))
    gamma = np.asarray(gamma, dtype=np.float32)
    Wq = np.asarray(Wq, dtype=np.float32)
    Wkv = np.asarray(Wkv, dtype=np.float32)
    q_scale = np.asarray(q_scale, dtype=np.float32)
    k_scale = np.asarray(k_scale, dtype=np.float32)
    Wo = np.ascontiguousarray(np.asarray(Wo, dtype=np.float32))
    rel_pos_bias = np.asarray(rel_pos_bias, dtype=np.float32)
    mask = np.asarray(mask)
    mask_ones = bool(mask.all())

    if PROFILE["enabled"]:
        _install_profile_hook()
    akey = ("a", mask_ones)
    if akey not in _cache:
        _cache[akey] = _build_launch_a(mask_ones)
    if "b" not in _cache:
        _cache["b"] = _build_launch_b()

    BF = ml_dtypes.bfloat16
    # host-side prep: transpose/cast x, LN stats, gamma-folded weights,
    # exponentiated transposed positional bias
    xT = np.ascontiguousarray(x.transpose(0, 2, 1)).astype(BF)
    mu = x.mean(-1)
    var = x.var(-1)
    numu = (-mu).astype(BF)                                   # [B, N]
    rstd = (1.0 / np.sqrt(var + LN_EPS)).astype(BF)           # [B, N]
    kb = np.where(mask, 0.0, NEG).astype(np.float32)

    wq_f = gamma[:, None] * Wq
    wk_f = gamma[:, None] * Wkv[:, :DIM]
    wv_f = gamma[:, None] * Wkv[:, DIM:]

    sblk4q = np.zeros((4, 128), np.float32)
    sblk4q[0, 0:64] = q_scale * 8.0
    sblk4q[1, 64:128] = q_scale * 8.0
    sblk4k = np.zeros((4, 128), np.float32)
    sblk4k[2, 0:64] = k_scale
    sblk4k[3, 64:128] = k_scale

    # B = exp(rel_pos_bias^T) in bf16, interleaved [jt, h, p, i] per core
    rpbT = rel_pos_bias.transpose(0, 2, 1)                     # [H, j, i]
    BE = np.exp(rpbT)

    in_maps_a = []
    for c in range(NCORES):
        es = slice(EH * c, EH * (c + 1))
        wq_s = np.ascontiguousarray(wq_f[:, es]).astype(BF)
        wk_s = np.ascontiguousarray(wk_f[:, es]).astype(BF)
        wv_s = np.ascontiguousarray(wv_f[:, es]).astype(BF)
        bint = np.ascontiguousarray(
            BE[2 * c:2 * c + 2].reshape(2, NJT, 128, N).transpose(1, 0, 2, 3)
        ).astype(BF)
        m = {
            "xT": xT,
            "rpbE": bint,
            "wq": wq_s, "wk": wk_s, "wv": wv_s,
            "csq": wq_s.astype(np.float32).sum(0)[None, :].astype(BF),
            "csk": wk_s.astype(np.float32).sum(0)[None, :].astype(BF),
            "csv": wv_s.astype(np.float32).sum(0)[None, :].astype(BF),
            "numu": numu, "rstd": rstd,
            "sblk4q": sblk4q, "sblk4k": sblk4k,
        }
        if not mask_ones:
            m["kb"] = kb
        in_maps_a.append(m)
    res_a = run_bass_kernel_spmd(_cache[akey], in_maps_a, list(range(NCORES)),
                                 trace=PROFILE["enabled"])
    if PROFILE["enabled"]:
        PROFILE["a_ns"] = res_a.exec_time_ns

    AT = np.empty((B, DIM, N), np.float32)
    S = np.empty((B, HEADS, N), np.float32)
    for c in range(NCORES):
        ao = res_a.results[c]["at_out"]            # [B, 2, 65, N]
        for h in range(2):
            AT[:, EH * c + 64 * h:EH * c + 64 * h + 64, :] = ao[:, h, 0:64, :]
            S[:, 2 * c + h, :] = ao[:, h, 64, :]
    AT_bf = AT.astype(BF)
    Wo_bf = Wo.astype(BF)

    sel = np.zeros((NCT, HEADS, 128), np.float32)
    for ct in range(NCT):
        sel[ct, 2 * ct, 0:64] = 1.0
        sel[ct, 2 * ct + 1, 64:128] = 1.0

    in_maps_b = []
    for c in range(NCORES):
        bi, ic = c // NIC, c % NIC
        in_maps_b.append({
            "a_t": np.ascontiguousarray(AT_bf[bi][:, ic * IC:(ic + 1) * IC]),
            "s_slice": np.ascontiguousarray(S[bi][:, ic * IC:(ic + 1) * IC]),
            "sel": sel,
            "wo": Wo_bf,
        })
    res_b = run_bass_kernel_spmd(_cache["b"], in_maps_b, list(range(NCORES)),
                                 trace=PROFILE["enabled"])
    if PROFILE["enabled"]:
        PROFILE["b_ns"] = res_b.exec_time_ns

    out = np.empty((B, N, DIM), np.float32)
    for c in range(NCORES):
        bi, ic = c // NIC, c % NIC
        out[bi, ic * IC:(ic + 1) * IC, :] = res_b.results[c]["out_rows"].astype(np.float32)
    return out


# revision 24
# speedup vs baseline: 1.6390x; 1.0111x over previous
"""Trainium2 Bass kernel for nn_Attention (2-batch, 16-head, n=2048, d=64 causal
attention with LayerNorm-projected l2-normalized q/k, relative position bias,
and output projection), SPMD across 8 NeuronCores.

Sharding: launch A tensor-parallels the 16 heads (2 heads per core, both
batches on every core) and emits transposed attention outputs; launch B
row-shards the final @ Wo matmul across the 8 cores.

Key structure (v2):
 - LayerNorm stats (mean/var) computed on host; gamma folded into the
   projection weights on host; the mean subtraction is a rank-1 matmul
   accumulation; rstd cancels in q/k l2norm and is applied to v.
 - rel_pos_bias enters multiplicatively: host precomputes B = exp(bias^T)
   in bf16, device computes E = exp(sim) straight out of PSUM (one wide
   activation over 4 PSUM banks = 2 j-tiles x 2 heads), then E*B on
   DVE/GpSimd in bf16.  Causal masking = affine_select fill 0.0 on B.
 - sim matmuls for the 2 heads are emitted as adjacent row-tiled (K=64)
   pairs at PE tile positions (0,0)/(64,0) so they can overlap.
 - attn@v uses a 65-wide v||ones stationary; row 64 carries softmax
   denominators; launch B normalizes and row-shards @ Wo in bf16.
 - phase 1 of batch 1 is software-pipelined into phase 2 of batch 0 to
   keep the tensor engine busy during the Act-bound softmax stretches.
"""

import numpy as np

HEADS = 16
DH = 64
B = 2
N = 2048
DIM = 1024
EH = 128          # per-core slice of the inner dim (2 heads x 64)
NCORES = 8
IC = 512          # i-chunk width
NIC = N // IC     # 4 i-chunks
JT = 128          # j-tile width
NJT = N // JT     # 16 j-tiles
NCT = DIM // 128  # 8 contraction tiles
LN_EPS = 1e-5
NEG = -1e30

_cache = {}


def _build_launch_a(mask_ones=True):
    import concourse.bass as bass
    import concourse.tile as tile
    from concourse import bacc, mybir
    from concourse.masks import make_identity

    F32 = mybir.dt.float32
    F32R = mybir.dt.float32r
    BF16 = mybir.dt.bfloat16
    AF = mybir.ActivationFunctionType
    nc = bacc.Bacc(None)
    # all large inputs are host-pre-laid-out so each DMA is one contiguous
    # multi-KB run per partition (descriptor-count, not bandwidth, limits
    # the DMA queues)
    xr_d = nc.declare_dram_parameter("xr", [B, NIC, 128, NCT, IC], BF16, isOutput=False)
    bc_d = [nc.declare_dram_parameter(f"bc{ic}", [128, 4 * (ic + 1), 2, IC],
                                      BF16, isOutput=False) for ic in range(NIC)]
    wq_d = nc.declare_dram_parameter("wq", [128, NCT, EH], BF16, isOutput=False)
    wk_d = nc.declare_dram_parameter("wk", [128, NCT, EH], BF16, isOutput=False)
    wv_d = nc.declare_dram_parameter("wv", [128, NCT, EH], BF16, isOutput=False)
    csq_d = nc.declare_dram_parameter("csq", [1, EH], BF16, isOutput=False)
    csk_d = nc.declare_dram_parameter("csk", [1, EH], BF16, isOutput=False)
    csv_d = nc.declare_dram_parameter("csv", [1, EH], BF16, isOutput=False)
    numu_d = nc.declare_dram_parameter("numu", [B, N], BF16, isOutput=False)
    rstd_d = nc.declare_dram_parameter("rstd", [B, N], BF16, isOutput=False)
    sbq_d = nc.declare_dram_parameter("sblk4q", [4, 128], F32, isOutput=False)
    sbk_d = nc.declare_dram_parameter("sblk4k", [4, 128], F32, isOutput=False)
    if not mask_ones:
        kb_d = nc.declare_dram_parameter("kb", [B, N], F32, isOutput=False)
    at_d = nc.declare_dram_parameter("at_out", [B, 2, 65, N], F32, isOutput=True)

    with tile.TileContext(nc) as tc:
        import contextlib
        with contextlib.ExitStack() as ctx:
            pers = ctx.enter_context(tc.tile_pool(name="pers", bufs=1))

            # ---------- constants ----------
            onescol_f = pers.tile([128, 1], F32, tag="onescol_f")
            nc.vector.memset(onescol_f, 1.0)
            row_f = pers.tile([1, 128], F32, tag="row_f")
            nc.vector.memset(row_f, 1.0)
            ones_row_bf = pers.tile([1, 128], BF16, tag="ones_row_bf")
            nc.vector.tensor_copy(out=ones_row_bf, in_=row_f)
            ident = pers.tile([128, 128], F32, tag="ident")
            make_identity(nc, ident)
            ident_bf = pers.tile([128, 128], BF16, tag="ident_bf")
            nc.vector.tensor_copy(out=ident_bf, in_=ident)
            eps4 = pers.tile([4, 1], F32, tag="eps4")
            nc.vector.memset(eps4, 1e-24)

            # ssq stationaries: o4q cols 0-1 head-blockdiag, o4k cols 2-3
            o4_f = pers.tile([128, 4], F32, tag="o4_f")
            nc.vector.memset(o4_f, 0.0)
            nc.vector.memset(o4_f[0:64, 0:1], 1.0)
            nc.vector.memset(o4_f[64:128, 1:2], 1.0)
            o4q = pers.tile([128, 4], BF16, tag="o4q")
            nc.vector.tensor_copy(out=o4q, in_=o4_f)
            nc.vector.memset(o4_f, 0.0)
            nc.vector.memset(o4_f[0:64, 2:3], 1.0)
            nc.vector.memset(o4_f[64:128, 3:4], 1.0)
            o4k = pers.tile([128, 4], BF16, tag="o4k")
            nc.vector.tensor_copy(out=o4k, in_=o4_f)

            # scale-broadcast stationaries (f32r)
            sbq_f = pers.tile([4, 128], F32, tag="sbq_f")
            nc.sync.dma_start(out=sbq_f, in_=sbq_d.ap())
            sbq_r = pers.tile([4, 128], F32R, tag="sbq_r")
            nc.vector.tensor_copy(out=sbq_r, in_=sbq_f)
            sbk_f = pers.tile([4, 128], F32, tag="sbk_f")
            nc.sync.dma_start(out=sbk_f, in_=sbk_d.ap())
            sbk_r = pers.tile([4, 128], F32R, tag="sbk_r")
            nc.vector.tensor_copy(out=sbk_r, in_=sbk_f)

            # weights (host gamma-folded), correction rows, LN stats rows
            wps = {}
            css = {}
            for nm, wd, cd in (("q", wq_d, csq_d), ("k", wk_d, csk_d),
                               ("v", wv_d, csv_d)):
                wp = pers.tile([128, NCT, EH], BF16, tag=f"w{nm}p", name=f"wp{nm}")
                nc.sync.dma_start(out=wp, in_=wd.ap())
                cs = pers.tile([1, EH], BF16, tag=f"cs{nm}", name=f"cs{nm}")
                nc.sync.dma_start(out=cs, in_=cd.ap())
                wps[nm] = wp
                css[nm] = cs
            numu_sb = pers.tile([1, B, N], BF16, tag="numu_sb")
            nc.sync.dma_start(out=numu_sb, in_=numu_d.ap().unsqueeze(0))
            rstd_sb = pers.tile([1, B, N], BF16, tag="rstd_sb")
            nc.sync.dma_start(out=rstd_sb, in_=rstd_d.ap().unsqueeze(0))
            if not mask_ones:
                kbT = pers.tile([128, B, NJT], F32, tag="kbT")
                nc.sync.dma_start(out=kbT, in_=kb_d.ap().rearrange("b (t p) -> p b t", p=128))

            # persistent per-batch products
            qhat = [pers.tile([128, N], BF16, tag=f"qhat{b}", name=f"qhat{b}") for b in range(B)]
            khat = [pers.tile([128, N], BF16, tag=f"khat{b}", name=f"khat{b}") for b in range(B)]
            v_all = [pers.tile([128, NJT, 130], BF16, tag=f"vall{b}", name=f"vall{b}") for b in range(B)]
            for b in range(B):
                for jt in range(NJT):
                    nc.vector.tensor_copy(out=v_all[b][:, jt, 64:65], in_=onescol_f)
                    nc.vector.tensor_copy(out=v_all[b][:, jt, 129:130], in_=onescol_f)

            # ---------- pools ----------
            sim_ps = ctx.enter_context(tc.tile_pool(name="sim_ps", bufs=2, space="PSUM"))
            av_ps = ctx.enter_context(tc.tile_pool(name="av_ps", bufs=2, space="PSUM"))
            mix_ps = ctx.enter_context(tc.tile_pool(name="mix_ps", bufs=2, space="PSUM"))
            xr_pool = ctx.enter_context(tc.tile_pool(name="xr_pool", bufs=2))
            bc_pool = ctx.enter_context(tc.tile_pool(name="bc_pool", bufs=2))
            e_pool = ctx.enter_context(tc.tile_pool(name="e_pool", bufs=2))
            m_pool = ctx.enter_context(tc.tile_pool(name="m_pool", bufs=2))
            sq_pool = ctx.enter_context(tc.tile_pool(name="sq_pool", bufs=2))
            rn_pool = ctx.enter_context(tc.tile_pool(name="rn_pool", bufs=1))
            ssq_pool = ctx.enter_context(tc.tile_pool(name="ssq_pool", bufs=1))
            rnr_pool = ctx.enter_context(tc.tile_pool(name="rnr_pool", bufs=1))
            raw_pool = ctx.enter_context(tc.tile_pool(name="raw_pool", bufs=8))
            vsc_pool = ctx.enter_context(tc.tile_pool(name="vsc_pool", bufs=2))
            stg_pool = ctx.enter_context(tc.tile_pool(name="stg_pool", bufs=2))

            # ---------- phase-1 work units ----------
            def ph1_units(b):
                """Emission closures for LN+proj+l2norm of one batch.
                Pass 1 per chunk: projections, squares, ssq; then ONE
                clustered Rsqrt over all chunks (avoids act-table thrash
                with the attention Exp), then per-chunk scale+hat mults."""
                units = []
                state = {}
                ssq_all = ssq_pool.tile([4, NIC, IC], F32, tag="ssqall",
                                        name=f"ssqall{b}")
                rn_r = rnr_pool.tile([4, N], F32R, tag="rnr", name=f"rnr{b}")
                for ic in range(NIC):
                    isl = slice(ic * IC, (ic + 1) * IC)

                    def u_load(b=b, ic=ic, isl=isl):
                        xr = xr_pool.tile([128, NCT, IC], BF16, tag="xr", name="xr")
                        nc.sync.dma_start(out=xr, in_=xr_d.ap()[b, ic])
                        state[ic] = {"xr": xr}
                    units.append(u_load)

                    def mk_proj(nm, b=b, ic=ic, isl=isl):
                        def u_proj_a():
                            st = state[ic]
                            pp = mix_ps.tile([128, IC], F32, tag="mx", name=f"pp{nm}")
                            for ct in range(4):
                                nc.tensor.matmul(pp, wps[nm][:, ct, :], st["xr"][:, ct, :],
                                                 start=(ct == 0), stop=False)
                            st[f"pp{nm}"] = pp
                        def u_proj_b():
                            st = state[ic]
                            pp = st[f"pp{nm}"]
                            for ct in range(4, NCT):
                                nc.tensor.matmul(pp, wps[nm][:, ct, :], st["xr"][:, ct, :],
                                                 start=False, stop=False)
                            nc.tensor.matmul(pp, css[nm], numu_sb[0:1, b, isl],
                                             start=False, stop=True)
                        return u_proj_a, u_proj_b
                    qa, qb = mk_proj("q")
                    ka, kb_ = mk_proj("k")
                    va, vb = mk_proj("v")

                    def u_qpost(b=b, ic=ic):
                        st = state[ic]
                        q_raw = raw_pool.tile([128, IC], BF16, tag="raw", name="q_raw")
                        nc.vector.tensor_copy(out=q_raw, in_=st["ppq"])
                        sq_q = sq_pool.tile([128, IC], BF16, tag="sq", name="sq_q")
                        nc.vector.tensor_mul(sq_q, q_raw, q_raw)
                        st["q_raw"] = q_raw
                        st["sq_q"] = sq_q

                    def u_kpost(b=b, ic=ic):
                        st = state[ic]
                        k_raw = raw_pool.tile([128, IC], BF16, tag="raw", name="k_raw")
                        nc.vector.tensor_copy(out=k_raw, in_=st["ppk"])
                        sq_k = sq_pool.tile([128, IC], BF16, tag="sq", name="sq_k")
                        nc.vector.tensor_mul(sq_k, k_raw, k_raw)
                        st["k_raw"] = k_raw
                        st["sq_k"] = sq_k

                    def u_ssq(b=b, ic=ic):
                        st = state[ic]
                        ssq4 = mix_ps.tile([4, IC], F32, tag="mx", name="ssq4")
                        nc.tensor.matmul(ssq4, o4q, st["sq_q"], start=True, stop=False)
                        nc.tensor.matmul(ssq4, o4k, st["sq_k"], start=False, stop=True)
                        nc.vector.tensor_copy(out=ssq_all[:, ic, :], in_=ssq4)

                    def u_vpost(b=b, ic=ic, isl=isl):
                        st = state[ic]
                        rstdb = mix_ps.tile([128, IC], F32, tag="mx", name="rstdb")
                        nc.tensor.matmul(rstdb, ones_row_bf, rstd_sb[0:1, b, isl],
                                         start=True, stop=True)
                        rb_sb = vsc_pool.tile([128, IC], F32, tag="rb", name="rb_sb")
                        nc.vector.tensor_copy(out=rb_sb, in_=rstdb)
                        vsc = vsc_pool.tile([128, IC], BF16, tag="vsc", name="vsc")
                        nc.vector.tensor_mul(vsc, st["ppv"], rb_sb)
                        st["vsc"] = vsc

                    def u_vtrans(b=b, ic=ic):
                        st = state[ic]
                        for k in range(IC // 128):
                            jt = ic * (IC // 128) + k
                            vt = mix_ps.tile([128, 128], BF16, tag="mx", name="vt")
                            nc.tensor.transpose(vt, st["vsc"][:, k * 128:(k + 1) * 128], ident_bf)
                            nc.vector.tensor_copy(out=v_all[b][:, jt, 0:64], in_=vt[:, 0:64])
                            nc.vector.tensor_copy(out=v_all[b][:, jt, 65:129], in_=vt[:, 64:128])

                    units += [qa, qb, u_qpost, ka, kb_, u_kpost, u_ssq,
                              va, vb, u_vpost, u_vtrans]

                def u_rsqrt(b=b):
                    rec = rn_pool.tile([4, N], F32, tag="rn", name="rec")
                    nc.vector.reciprocal_approx_fast(out=rec, in_=ssq_all)
                    nc.scalar.activation(out=rn_r, in_=rec, func=AF.Sqrt)
                units.append(u_rsqrt)

                for ic in range(NIC):
                    isl = slice(ic * IC, (ic + 1) * IC)

                    def u_hats(b=b, ic=ic, isl=isl):
                        st = state[ic]
                        sr_q = mix_ps.tile([128, IC], F32, tag="mx", name="sr_q")
                        nc.tensor.matmul(sr_q, sbq_r, rn_r[:, isl], start=True, stop=True)
                        nc.vector.tensor_mul(qhat[b][:, isl], st["q_raw"], sr_q)
                        sr_k = mix_ps.tile([128, IC], F32, tag="mx", name="sr_k")
                        nc.tensor.matmul(sr_k, sbk_r, rn_r[:, isl], start=True, stop=True)
                        nc.vector.tensor_mul(khat[b][:, isl], st["k_raw"], sr_k)
                        del state[ic]
                    units.append(u_hats)
                return units

            # ---------- phase-2 (attention) ----------
            def load_bias_chunk(ic):
                jmax = (IC // 128) * (ic + 1)
                isl = slice(ic * IC, (ic + 1) * IC)
                Bc = bc_pool.tile([128, jmax, 2, IC], BF16, tag="bc", name="Bc",
                                  padded_shape=[128, NJT, 2, IC])
                nc.sync.dma_start(out=Bc, in_=bc_d[ic].ap())
                # causal mask: zero B above the diagonal, in place, trimmed to
                # the valid suffix (the masked prefix is never read by attn@v)
                for k in range(4):
                    w = IC - 128 * k
                    for h in range(2):
                        nc.gpsimd.affine_select(
                            out=Bc[:, jmax - 4 + k, h, 128 * k:],
                            in_=Bc[:, jmax - 4 + k, h, 128 * k:],
                            compare_op=mybir.AluOpType.is_ge,
                            fill=0.0, base=0, channel_multiplier=-1,
                            pattern=[[1, w]])
                return Bc

            def ph2_chunk(b, ic, Bc, feed):
                """feed: list of ph1 unit closures to interleave between groups."""
                jmax = (IC // 128) * (ic + 1)
                isl = slice(ic * IC, (ic + 1) * IC)
                avs = [av_ps.tile([65, IC], F32, tag="av", name=f"av{h}")
                       for h in range(2)]
                diag0 = jmax - 4
                for jt in range(jmax):
                    # causal trim: diag j-tiles only need i >= jt*128
                    off = max(0, (jt - diag0) * 128)
                    sp = sim_ps.tile([128, 2, IC], F32, tag="sp", name="sp")
                    for h in range(2):
                        dsl = slice(64 * h, 64 * h + 64)
                        nc.tensor.matmul(
                            sp[:, h, off:],
                            khat[b][dsl, jt * 128:(jt + 1) * 128],
                            qhat[b][dsl, isl.start + off:isl.stop],
                            start=True, stop=True)
                    E = e_pool.tile([128, 2, IC], BF16, tag="E", name="E")
                    if mask_ones:
                        nc.scalar.activation(out=E, in_=sp, func=AF.Exp)
                    else:
                        for h in range(2):
                            nc.scalar.activation(out=E[:, h, :],
                                                 in_=sp[:, h, :],
                                                 func=AF.Exp,
                                                 bias=kbT[:, b, jt:jt + 1])
                    Em = m_pool.tile([128, 2, IC], BF16, tag="Em", name="Em")
                    nc.vector.tensor_mul(Em, E, Bc[:, jt, :, :])
                    for h in range(2):
                        nc.tensor.matmul(
                            avs[h][:, off:], v_all[b][:, jt, 65 * h:65 * h + 65],
                            Em[:, h, off:],
                            start=(jt == 0), stop=(jt == jmax - 1))
                    # software-pipeline phase-1 work of the other batch
                    if feed:
                        feed.pop(0)()
                for h in range(2):
                    stg = stg_pool.tile([65, IC], F32, tag="stg", name="stg")
                    nc.vector.tensor_copy(out=stg, in_=avs[h][0:65, :])
                    nc.sync.dma_start(out=at_d.ap()[b, h, :, isl], in_=stg)

            # ---------- main schedule ----------
            for u in ph1_units(0):
                u()
            # zero the sim psum banks once so trimmed regions never hold
            # unbounded garbage (exp of it must stay finite)
            for i in range(2):
                sp0 = sim_ps.tile([128, 2, IC], F32, tag="sp", name="sp0")
                nc.vector.memset(sp0, 0.0)
            feed = ph1_units(1)
            # phase 2: all of b0 (absorbing ph1(b1) between groups), then b1
            for b in range(B):
                if b == 1:
                    while feed:
                        feed.pop(0)()
                for ic in range(NIC):
                    Bc = load_bias_chunk(ic)
                    ph2_chunk(b, ic, Bc, feed if b == 0 else [])
    nc.compile()
    return nc


def _build_launch_b():
    import concourse.bass as bass
    import concourse.tile as tile
    from concourse import bacc, mybir

    F32 = mybir.dt.float32
    F32R = mybir.dt.float32r
    BF16 = mybir.dt.bfloat16

    nc = bacc.Bacc(None)
    at_d = nc.declare_dram_parameter("a_t", [DIM, IC], BF16, isOutput=False)
    s_d = nc.declare_dram_parameter("s_slice", [HEADS, IC], F32, isOutput=False)
    sel_d = nc.declare_dram_parameter("sel", [NCT, HEADS, 128], F32, isOutput=False)
    wo_d = nc.declare_dram_parameter("wo", [DIM, DIM], BF16, isOutput=False)
    out_d = nc.declare_dram_parameter("out_rows", [IC, DIM], BF16, isOutput=True)

    with tile.TileContext(nc) as tc:
        with tc.tile_pool(name="sb", bufs=1) as sb, \
             tc.tile_pool(name="wl", bufs=2) as wl, \
             tc.tile_pool(name="ob", bufs=2) as ob, \
             tc.tile_pool(name="rb_ps", bufs=2, space="PSUM") as rb_ps, \
             tc.tile_pool(name="ps", bufs=2, space="PSUM") as ps:
            a_sb = sb.tile([128, NCT, IC], BF16, tag="a")
            nc.sync.dma_start(
                out=a_sb,
                in_=at_d.ap().rearrange("(t p) i -> p t i", p=128))
            s_sb = sb.tile([HEADS, IC], F32, tag="s")
            nc.sync.dma_start(out=s_sb, in_=s_d.ap())
            sel_sb = sb.tile([HEADS, NCT, 128], F32, tag="sel")
            nc.sync.dma_start(out=sel_sb, in_=sel_d.ap().rearrange("t h p -> h t p"))
            rs_f = sb.tile([HEADS, IC], F32, tag="rs_f")
            nc.vector.reciprocal_approx_fast(out=rs_f, in_=s_sb)
            rs_r = sb.tile([HEADS, IC], F32R, tag="rs_r")
            nc.vector.tensor_copy(out=rs_r, in_=rs_f)
            wo_sb = sb.tile([128, NCT, DIM], BF16, tag="wo")
            nc.sync.dma_start(
                out=wo_sb,
                in_=wo_d.ap().rearrange("(t p) o -> p t o", p=128))
            # normalized bf16 activations: a_n[c, i] = a[c, i] / s[head(c), i]
            a_n = sb.tile([128, NCT, IC], BF16, tag="a_n")
            for ct in range(NCT):
                selr = wl.tile([HEADS, 128], F32R, tag="selr")
                nc.vector.tensor_copy(out=selr, in_=sel_sb[:, ct, :])
                rsb = rb_ps.tile([128, IC], F32, tag="rsb")
                nc.tensor.matmul(rsb, selr, rs_r, start=True, stop=True)
                nc.vector.tensor_mul(a_n[:, ct, :], rsb, a_sb[:, ct, :])
            for m in range(IC // 128):
                osb = ob.tile([128, DIM], BF16, tag="osb")
                for oc in range(2):
                    pp = ps.tile([128, 512], F32, tag="pp")
                    for ct in range(NCT):
                        nc.tensor.matmul(
                            pp, a_n[:, ct, m * 128:(m + 1) * 128],
                            wo_sb[:, ct, oc * 512:(oc + 1) * 512],
                            start=(ct == 0), stop=(ct == NCT - 1))
                    nc.vector.tensor_copy(out=osb[:, oc * 512:(oc + 1) * 512], in_=pp)
                nc.sync.dma_start(out=out_d.ap()[m * 128:(m + 1) * 128, :], in_=osb)

    nc.compile()
    return nc


PROFILE = {"enabled": False, "a_ns": None, "b_ns": None}


def _install_profile_hook():
    """Register the axon NTFF profile hook (the image's antenv lacks
    axon_hooks, so run_bass_kernel_spmd(trace=True) would silently skip
    tracing).  Replicates trn_boot's ctypes recipe."""
    import sys, types, ctypes, contextlib

    if "antenv.axon_hooks" in sys.modules:
        return
    lib = ctypes.CDLL("/opt/axon/libaxon_pjrt.so")
    if not hasattr(lib, "axon_start_nrt_profile"):
        return
    lib.axon_start_nrt_profile.argtypes = [ctypes.POINTER(ctypes.c_int64), ctypes.c_size_t]
    lib.axon_start_nrt_profile.restype = ctypes.c_int64
    lib.axon_stop_nrt_profile.argtypes = [ctypes.c_char_p]
    lib.axon_stop_nrt_profile.restype = ctypes.c_int64

    @contextlib.contextmanager
    def _hook(output_dir, device_ids):
        import jax
        jax.devices()
        if device_ids:
            ids = (ctypes.c_int64 * len(device_ids))(*device_ids)
            rc = lib.axon_start_nrt_profile(ids, len(device_ids))
        else:
            rc = lib.axon_start_nrt_profile(None, 0)
        if rc != 0:
            raise RuntimeError(f"axon_start_nrt_profile rc={rc}")
        try:
            yield
        finally:
            n = lib.axon_stop_nrt_profile(str(output_dir).encode())
            print(f"profile: {n} file(s) written to {output_dir}")

    mod = types.ModuleType("antenv.axon_hooks")
    mod.get_axon_ntff_profile_hook = lambda: _hook
    mod.set_axon_ntff_profile_hook = lambda h: None
    sys.modules["antenv.axon_hooks"] = mod

    # avoid the S3 artifact upload inside the trace path
    from concourse import bass_utils
    bass_utils.upload_artifacts = lambda tmpdir: ""


def kernel(x, gamma, Wq, Wkv, q_scale, k_scale, Wo, rel_pos_bias, mask):
    from concourse.bass_utils import run_bass_kernel_spmd
    import ml_dtypes

    x = np.ascontiguousarray(np.asarray(x, dtype=np.float32))
    gamma = np.asarray(gamma, dtype=np.float32)
    Wq = np.asarray(Wq, dtype=np.float32)
    Wkv = np.asarray(Wkv, dtype=np.float32)
    q_scale = np.asarray(q_scale, dtype=np.float32)
    k_scale = np.asarray(k_scale, dtype=np.float32)
    Wo = np.ascontiguousarray(np.asarray(Wo, dtype=np.float32))
    rel_pos_bias = np.asarray(rel_pos_bias, dtype=np.float32)
    mask = np.asarray(mask)
    mask_ones = bool(mask.all())

    if PROFILE["enabled"]:
        _install_profile_hook()
    akey = ("a", mask_ones)
    if akey not in _cache:
        _cache[akey] = _build_launch_a(mask_ones)
    if "b" not in _cache:
        _cache["b"] = _build_launch_b()

    BF = ml_dtypes.bfloat16
    # host-side prep: transpose/cast x, LN stats, gamma-folded weights,
    # exponentiated transposed positional bias.  All large tensors are laid
    # out so device DMAs are partition-major contiguous.
    xT = x.transpose(0, 2, 1)                                  # [B, DIM, N]
    XR = np.ascontiguousarray(
        xT.reshape(B, NCT, 128, NIC, IC).transpose(0, 3, 2, 1, 4)).astype(BF)
    mu = x.mean(-1)
    var = x.var(-1)
    numu = (-mu).astype(BF)                                   # [B, N]
    rstd = (1.0 / np.sqrt(var + LN_EPS)).astype(BF)           # [B, N]
    kb = np.where(mask, 0.0, NEG).astype(np.float32)

    wq_f = gamma[:, None] * Wq
    wk_f = gamma[:, None] * Wkv[:, :DIM]
    wv_f = gamma[:, None] * Wkv[:, DIM:]

    sblk4q = np.zeros((4, 128), np.float32)
    sblk4q[0, 0:64] = q_scale * 8.0
    sblk4q[1, 64:128] = q_scale * 8.0
    sblk4k = np.zeros((4, 128), np.float32)
    sblk4k[2, 0:64] = k_scale
    sblk4k[3, 64:128] = k_scale

    # B = exp(rel_pos_bias^T) in bf16, per-chunk [p, jt, h, i] layout
    rpbT = rel_pos_bias.transpose(0, 2, 1)                     # [H, j, i]
    BE = np.exp(rpbT)

    def wlayout(w):
        # [DIM, EH] -> [128, NCT, EH] partition-major
        return np.ascontiguousarray(w.reshape(NCT, 128, EH).transpose(1, 0, 2))

    in_maps_a = []
    for c in range(NCORES):
        es = slice(EH * c, EH * (c + 1))
        wq_s = wlayout(wq_f[:, es]).astype(BF)
        wk_s = wlayout(wk_f[:, es]).astype(BF)
        wv_s = wlayout(wv_f[:, es]).astype(BF)
        # [h, jt, p, ic, i]
        bcore = BE[2 * c:2 * c + 2].reshape(2, NJT, 128, NIC, IC)
        m = {
            "xr": XR,
            "wq": wq_s, "wk": wk_s, "wv": wv_s,
            "csq": wq_s.astype(np.float32).sum((0, 1))[None, :].astype(BF),
            "csk": wk_s.astype(np.float32).sum((0, 1))[None, :].astype(BF),
            "csv": wv_s.astype(np.float32).sum((0, 1))[None, :].astype(BF),
            "numu": numu, "rstd": rstd,
            "sblk4q": sblk4q, "sblk4k": sblk4k,
        }
        for ic in range(NIC):
            jmax = 4 * (ic + 1)
            m[f"bc{ic}"] = np.ascontiguousarray(
                bcore[:, 0:jmax, :, ic, :].transpose(2, 1, 0, 3)).astype(BF)
        if not mask_ones:
            m["kb"] = kb
        in_maps_a.append(m)
    res_a = run_bass_kernel_spmd(_cache[akey], in_maps_a, list(range(NCORES)),
                                 trace=PROFILE["enabled"])
    if PROFILE["enabled"]:
        PROFILE["a_ns"] = res_a.exec_time_ns

    AT = np.empty((B, DIM, N), np.float32)
    S = np.empty((B, HEADS, N), np.float32)
    for c in range(NCORES):
        ao = res_a.results[c]["at_out"]            # [B, 2, 65, N]
        for h in range(2):
            AT[:, EH * c + 64 * h:EH * c + 64 * h + 64, :] = ao[:, h, 0:64, :]
            S[:, 2 * c + h, :] = ao[:, h, 64, :]
    AT_bf = AT.astype(BF)
    Wo_bf = Wo.astype(BF)

    sel = np.zeros((NCT, HEADS, 128), np.float32)
    for ct in range(NCT):
        sel[ct, 2 * ct, 0:64] = 1.0
        sel[ct, 2 * ct + 1, 64:128] = 1.0

    in_maps_b = []
    for c in range(NCORES):
        bi, ic = c // NIC, c % NIC
        in_maps_b.append({
            "a_t": np.ascontiguousarray(AT_bf[bi][:, ic * IC:(ic + 1) * IC]),
            "s_slice": np.ascontiguousarray(S[bi][:, ic * IC:(ic + 1) * IC]),
            "sel": sel,
            "wo": Wo_bf,
        })
    res_b = run_bass_kernel_spmd(_cache["b"], in_maps_b, list(range(NCORES)),
                                 trace=PROFILE["enabled"])
    if PROFILE["enabled"]:
        PROFILE["b_ns"] = res_b.exec_time_ns

    out = np.empty((B, N, DIM), np.float32)
    for c in range(NCORES):
        bi, ic = c // NIC, c % NIC
        out[bi, ic * IC:(ic + 1) * IC, :] = res_b.results[c]["out_rows"].astype(np.float32)
    return out


# revision 32
# speedup vs baseline: 1.7383x; 1.0606x over previous
"""Trainium2 Bass kernel for nn_Attention (2-batch, 16-head, n=2048, d=64 causal
attention with LayerNorm-projected l2-normalized q/k, relative position bias,
and output projection), SPMD across 8 NeuronCores.

Sharding: launch A tensor-parallels the 16 heads (2 heads per core, both
batches on every core) and emits transposed attention outputs; launch B
row-shards the final @ Wo matmul across the 8 cores.

Key structure (v2):
 - LayerNorm stats (mean/var) computed on host; gamma folded into the
   projection weights on host; the mean subtraction is a rank-1 matmul
   accumulation; rstd cancels in q/k l2norm and is applied to v.
 - rel_pos_bias enters multiplicatively: host precomputes B = exp(bias^T)
   in bf16, device computes E = exp(sim) straight out of PSUM (one wide
   activation over 4 PSUM banks = 2 j-tiles x 2 heads), then E*B on
   DVE/GpSimd in bf16.  Causal masking = affine_select fill 0.0 on B.
 - sim matmuls for the 2 heads are emitted as adjacent row-tiled (K=64)
   pairs at PE tile positions (0,0)/(64,0) so they can overlap.
 - attn@v uses a 65-wide v||ones stationary; row 64 carries softmax
   denominators; launch B normalizes and row-shards @ Wo in bf16.
 - phase 1 of batch 1 is software-pipelined into phase 2 of batch 0 to
   keep the tensor engine busy during the Act-bound softmax stretches.
"""

import numpy as np

HEADS = 16
DH = 64
B = 2
N = 2048
DIM = 1024
EH = 128          # per-core slice of the inner dim (2 heads x 64)
NCORES = 8
IC = 512          # i-chunk width
NIC = N // IC     # 4 i-chunks
JT = 128          # j-tile width
NJT = N // JT     # 16 j-tiles
NCT = DIM // 128  # 8 contraction tiles
LN_EPS = 1e-5
NEG = -1e30

_cache = {}


def _build_launch_a(mask_ones=True):
    import concourse.bass as bass
    import concourse.tile as tile
    from concourse import bacc, mybir
    from concourse.masks import make_identity

    F32 = mybir.dt.float32
    F32R = mybir.dt.float32r
    BF16 = mybir.dt.bfloat16
    AF = mybir.ActivationFunctionType
    nc = bacc.Bacc(None)
    # all large inputs are host-pre-laid-out so each DMA is one contiguous
    # multi-KB run per partition (descriptor-count, not bandwidth, limits
    # the DMA queues)
    F8 = None  # set below
    from concourse import mybir as _mb
    F8 = _mb.dt.float8e4
    xr_d = nc.declare_dram_parameter("xr", [B, NIC, 128, NCT, IC], BF16, isOutput=False)
    bc_d = [nc.declare_dram_parameter(f"bc{ic}", [128, 4 * (ic + 1), 2, IC],
                                      BF16, isOutput=False) for ic in range(NIC)]
    wq_d = nc.declare_dram_parameter("wq", [128, NCT, EH], BF16, isOutput=False)
    wk_d = nc.declare_dram_parameter("wk", [128, NCT, EH], BF16, isOutput=False)
    wv_d = nc.declare_dram_parameter("wv", [128, NCT, EH], BF16, isOutput=False)
    csq_d = nc.declare_dram_parameter("csq", [1, EH], BF16, isOutput=False)
    csk_d = nc.declare_dram_parameter("csk", [1, EH], BF16, isOutput=False)
    csv_d = nc.declare_dram_parameter("csv", [1, EH], BF16, isOutput=False)
    numu_d = nc.declare_dram_parameter("numu", [B, N], BF16, isOutput=False)
    sbq_d = nc.declare_dram_parameter("sblk4q", [4, 128], F32, isOutput=False)
    sbk_d = nc.declare_dram_parameter("sblk4k", [4, 128], F32, isOutput=False)
    if not mask_ones:
        kb_d = nc.declare_dram_parameter("kb", [B, N], F32, isOutput=False)
    at_d = nc.declare_dram_parameter("at_out", [B, 2, 65, N], F32, isOutput=True)

    with tile.TileContext(nc) as tc:
        import contextlib
        with contextlib.ExitStack() as ctx:
            pers = ctx.enter_context(tc.tile_pool(name="pers", bufs=1))

            # ---------- constants ----------
            onescol_f = pers.tile([128, 1], F32, tag="onescol_f")
            nc.vector.memset(onescol_f, 1.0)
            row_f = pers.tile([1, 128], F32, tag="row_f")
            nc.vector.memset(row_f, 1.0)
            ones_row_bf = pers.tile([1, 128], BF16, tag="ones_row_bf")
            nc.vector.tensor_copy(out=ones_row_bf, in_=row_f)
            ident = pers.tile([128, 128], F32, tag="ident")
            make_identity(nc, ident)
            ident_bf = pers.tile([128, 128], BF16, tag="ident_bf")
            nc.vector.tensor_copy(out=ident_bf, in_=ident)
            eps4 = pers.tile([4, 1], F32, tag="eps4")
            nc.vector.memset(eps4, 1e-24)

            # ssq stationaries: o4q cols 0-1 head-blockdiag, o4k cols 2-3
            o4_f = pers.tile([128, 4], F32, tag="o4_f")
            nc.vector.memset(o4_f, 0.0)
            nc.vector.memset(o4_f[0:64, 0:1], 1.0)
            nc.vector.memset(o4_f[64:128, 1:2], 1.0)
            o4q = pers.tile([128, 4], BF16, tag="o4q")
            nc.vector.tensor_copy(out=o4q, in_=o4_f)
            nc.vector.memset(o4_f, 0.0)
            nc.vector.memset(o4_f[0:64, 2:3], 1.0)
            nc.vector.memset(o4_f[64:128, 3:4], 1.0)
            o4k = pers.tile([128, 4], BF16, tag="o4k")
            nc.vector.tensor_copy(out=o4k, in_=o4_f)

            # scale-broadcast stationaries (f32r)
            sbq_f = pers.tile([4, 128], F32, tag="sbq_f")
            nc.sync.dma_start(out=sbq_f, in_=sbq_d.ap())
            sbq_r = pers.tile([4, 128], F32R, tag="sbq_r")
            nc.vector.tensor_copy(out=sbq_r, in_=sbq_f)
            sbk_f = pers.tile([4, 128], F32, tag="sbk_f")
            nc.sync.dma_start(out=sbk_f, in_=sbk_d.ap())
            sbk_r = pers.tile([4, 128], F32R, tag="sbk_r")
            nc.vector.tensor_copy(out=sbk_r, in_=sbk_f)

            # weights (host gamma-folded), correction rows, LN stats rows
            wps = {}
            css = {}
            for nm, wd, cd in (("q", wq_d, csq_d), ("k", wk_d, csk_d),
                               ("v", wv_d, csv_d)):
                wp = pers.tile([128, NCT, EH], BF16, tag=f"w{nm}p", name=f"wp{nm}")
                nc.sync.dma_start(out=wp, in_=wd.ap())
                cs = pers.tile([1, EH], BF16, tag=f"cs{nm}", name=f"cs{nm}")
                nc.sync.dma_start(out=cs, in_=cd.ap())
                wps[nm] = wp
                css[nm] = cs
            numu_sb = pers.tile([1, B, N], BF16, tag="numu_sb")
            nc.sync.dma_start(out=numu_sb, in_=numu_d.ap().unsqueeze(0))
            if not mask_ones:
                kbT = pers.tile([128, B, NJT], F32, tag="kbT")
                nc.sync.dma_start(out=kbT, in_=kb_d.ap().rearrange("b (t p) -> p b t", p=128))

            # persistent per-batch products
            qhat = [pers.tile([128, N], BF16, tag=f"qhat{b}", name=f"qhat{b}") for b in range(B)]
            khat = [pers.tile([128, N], BF16, tag=f"khat{b}", name=f"khat{b}") for b in range(B)]
            v_all = [pers.tile([128, NJT, 130], BF16, tag=f"vall{b}", name=f"vall{b}") for b in range(B)]
            for b in range(B):
                for jt in range(NJT):
                    nc.vector.tensor_copy(out=v_all[b][:, jt, 64:65], in_=onescol_f)
                    nc.vector.tensor_copy(out=v_all[b][:, jt, 129:130], in_=onescol_f)

            # ---------- pools ----------
            sim_ps = ctx.enter_context(tc.tile_pool(name="sim_ps", bufs=2, space="PSUM"))
            av_ps = ctx.enter_context(tc.tile_pool(name="av_ps", bufs=2, space="PSUM"))
            mix_ps = ctx.enter_context(tc.tile_pool(name="mix_ps", bufs=2, space="PSUM"))
            xr_pool = ctx.enter_context(tc.tile_pool(name="xr_pool", bufs=2))
            bc_pool = ctx.enter_context(tc.tile_pool(name="bc_pool", bufs=2))
            e_pool = ctx.enter_context(tc.tile_pool(name="e_pool", bufs=2))
            m_pool = ctx.enter_context(tc.tile_pool(name="m_pool", bufs=2))
            sq_pool = ctx.enter_context(tc.tile_pool(name="sq_pool", bufs=2))
            rn_pool = ctx.enter_context(tc.tile_pool(name="rn_pool", bufs=1))
            ssq_pool = ctx.enter_context(tc.tile_pool(name="ssq_pool", bufs=1))
            rnr_pool = ctx.enter_context(tc.tile_pool(name="rnr_pool", bufs=1))
            raw_pool = ctx.enter_context(tc.tile_pool(name="raw_pool", bufs=8))
            vsc_pool = ctx.enter_context(tc.tile_pool(name="vsc_pool", bufs=2))
            stg_pool = ctx.enter_context(tc.tile_pool(name="stg_pool", bufs=2))

            # ---------- phase-1 work units ----------
            def ph1_units(b):
                """Emission closures for LN+proj+l2norm of one batch.
                Pass 1 per chunk: projections, squares, ssq; then ONE
                clustered Rsqrt over all chunks (avoids act-table thrash
                with the attention Exp), then per-chunk scale+hat mults."""
                units = []
                state = {}
                ssq_all = ssq_pool.tile([4, NIC, IC], F32, tag="ssqall",
                                        name=f"ssqall{b}")
                rn_r = rnr_pool.tile([4, N], F32R, tag="rnr", name=f"rnr{b}")
                for ic in range(NIC):
                    isl = slice(ic * IC, (ic + 1) * IC)

                    def u_load(b=b, ic=ic, isl=isl):
                        xr = xr_pool.tile([128, NCT, IC], BF16, tag="xr", name="xr")
                        nc.sync.dma_start(out=xr, in_=xr_d.ap()[b, ic])
                        state[ic] = {"xr": xr}
                    units.append(u_load)

                    def mk_proj(nm, b=b, ic=ic, isl=isl):
                        def u_proj_a():
                            st = state[ic]
                            pp = mix_ps.tile([128, IC], F32, tag="mx", name=f"pp{nm}")
                            for ct in range(4):
                                nc.tensor.matmul(pp, wps[nm][:, ct, :], st["xr"][:, ct, :],
                                                 start=(ct == 0), stop=False)
                            st[f"pp{nm}"] = pp
                        def u_proj_b():
                            st = state[ic]
                            pp = st[f"pp{nm}"]
                            for ct in range(4, NCT):
                                nc.tensor.matmul(pp, wps[nm][:, ct, :], st["xr"][:, ct, :],
                                                 start=False, stop=False)
                            nc.tensor.matmul(pp, css[nm], numu_sb[0:1, b, isl],
                                             start=False, stop=True)
                        return u_proj_a, u_proj_b
                    qa, qb = mk_proj("q")
                    ka, kb_ = mk_proj("k")
                    va, vb = mk_proj("v")

                    def u_qpost(b=b, ic=ic):
                        st = state[ic]
                        q_raw = raw_pool.tile([128, IC], BF16, tag="raw", name="q_raw")
                        nc.vector.tensor_copy(out=q_raw, in_=st["ppq"])
                        sq_q = sq_pool.tile([128, IC], BF16, tag="sq", name="sq_q")
                        nc.vector.tensor_mul(sq_q, q_raw, q_raw)
                        st["q_raw"] = q_raw
                        st["sq_q"] = sq_q

                    def u_kpost(b=b, ic=ic):
                        st = state[ic]
                        k_raw = raw_pool.tile([128, IC], BF16, tag="raw", name="k_raw")
                        nc.vector.tensor_copy(out=k_raw, in_=st["ppk"])
                        sq_k = sq_pool.tile([128, IC], BF16, tag="sq", name="sq_k")
                        nc.vector.tensor_mul(sq_k, k_raw, k_raw)
                        st["k_raw"] = k_raw
                        st["sq_k"] = sq_k

                    def u_ssq(b=b, ic=ic):
                        st = state[ic]
                        ssq4 = mix_ps.tile([4, IC], F32, tag="mx", name="ssq4")
                        nc.tensor.matmul(ssq4, o4q, st["sq_q"], start=True, stop=False)
                        nc.tensor.matmul(ssq4, o4k, st["sq_k"], start=False, stop=True)
                        nc.vector.tensor_copy(out=ssq_all[:, ic, :], in_=ssq4)

                    def u_vfin(b=b, ic=ic):
                        # rstd is folded into x on the host, so v = ppv directly
                        st = state[ic]
                        vsc = vsc_pool.tile([128, IC], BF16, tag="vsc", name="vsc")
                        nc.vector.tensor_copy(out=vsc, in_=st["ppv"])
                        vtp = av_ps if b == 0 else mix_ps
                        vtag = "av" if b == 0 else "mx"
                        for k in range(IC // 128):
                            jt = ic * (IC // 128) + k
                            vt = vtp.tile([128, 128], BF16, tag=vtag, name="vt")
                            nc.tensor.transpose(vt, vsc[:, k * 128:(k + 1) * 128], ident_bf)
                            nc.vector.tensor_copy(out=v_all[b][:, jt, 0:64], in_=vt[:, 0:64])
                            nc.vector.tensor_copy(out=v_all[b][:, jt, 65:129], in_=vt[:, 64:128])

                    units += [va, vb, u_vfin, qa, qb, u_qpost, ka, kb_,
                              u_kpost, u_ssq]

                def u_rsqrt(b=b):
                    rec = rn_pool.tile([4, N], F32, tag="rn", name="rec")
                    nc.vector.reciprocal_approx_fast(out=rec, in_=ssq_all)
                    nc.scalar.activation(out=rn_r, in_=rec, func=AF.Sqrt)
                units.append(u_rsqrt)

                for ic in range(NIC):
                    isl = slice(ic * IC, (ic + 1) * IC)

                    def u_hats(b=b, ic=ic, isl=isl):
                        st = state[ic]
                        sr_q = mix_ps.tile([128, IC], F32, tag="mx", name="sr_q")
                        nc.tensor.matmul(sr_q, sbq_r, rn_r[:, isl], start=True, stop=True)
                        nc.vector.tensor_mul(qhat[b][:, isl], st["q_raw"], sr_q)
                        sr_k = mix_ps.tile([128, IC], F32, tag="mx", name="sr_k")
                        nc.tensor.matmul(sr_k, sbk_r, rn_r[:, isl], start=True, stop=True)
                        nc.vector.tensor_mul(khat[b][:, isl], st["k_raw"], sr_k)
                        del state[ic]
                    units.append(u_hats)
                return units

            # ---------- phase-2 (attention) ----------
            def load_bias_chunk(ic):
                jmax = (IC // 128) * (ic + 1)
                isl = slice(ic * IC, (ic + 1) * IC)
                Bc = bc_pool.tile([128, jmax, 2, IC], BF16, tag="bc", name="Bc",
                                  padded_shape=[128, NJT, 2, IC])
                nc.sync.dma_start(out=Bc, in_=bc_d[ic].ap())
                # causal mask: zero B above the diagonal, in place, trimmed to
                # the valid suffix (the masked prefix is never read by attn@v)
                for k in range(4):
                    w = IC - 128 * k
                    for h in range(2):
                        nc.gpsimd.affine_select(
                            out=Bc[:, jmax - 4 + k, h, 128 * k:],
                            in_=Bc[:, jmax - 4 + k, h, 128 * k:],
                            compare_op=mybir.AluOpType.is_ge,
                            fill=0.0, base=0, channel_multiplier=-1,
                            pattern=[[1, w]])
                return Bc

            def ph2_chunk(b, ic, Bc, feed):
                """feed: list of ph1 unit closures to interleave between groups."""
                jmax = (IC // 128) * (ic + 1)
                isl = slice(ic * IC, (ic + 1) * IC)
                avs = [av_ps.tile([65, IC], F32, tag="av", name=f"av{h}")
                       for h in range(2)]
                diag0 = jmax - 4
                for jt in range(jmax):
                    # causal trim: diag j-tiles only need i >= jt*128
                    off = max(0, (jt - diag0) * 128)
                    sp = sim_ps.tile([128, 2, IC], F32, tag="sp", name="sp")
                    for h in range(2):
                        dsl = slice(64 * h, 64 * h + 64)
                        nc.tensor.matmul(
                            sp[:, h, off:],
                            khat[b][dsl, jt * 128:(jt + 1) * 128],
                            qhat[b][dsl, isl.start + off:isl.stop],
                            start=True, stop=True)
                    E = e_pool.tile([128, 2, IC], BF16, tag="E", name="E")
                    if mask_ones:
                        nc.scalar.activation(out=E, in_=sp, func=AF.Exp)
                    else:
                        for h in range(2):
                            nc.scalar.activation(out=E[:, h, :],
                                                 in_=sp[:, h, :],
                                                 func=AF.Exp,
                                                 bias=kbT[:, b, jt:jt + 1])
                    Em = m_pool.tile([128, 2, IC], BF16, tag="Em", name="Em")
                    nc.vector.tensor_mul(Em, E, Bc[:, jt, :, :])
                    for h in range(2):
                        nc.tensor.matmul(
                            avs[h][:, off:], v_all[b][:, jt, 65 * h:65 * h + 65],
                            Em[:, h, off:],
                            start=(jt == 0), stop=(jt == jmax - 1))
                    # software-pipeline phase-1 work of the other batch
                    if feed:
                        feed.pop(0)()
                for h in range(2):
                    stg = stg_pool.tile([65, IC], F32, tag="stg", name="stg")
                    nc.vector.tensor_copy(out=stg, in_=avs[h][0:65, :])
                    nc.sync.dma_start(out=at_d.ap()[b, h, :, isl], in_=stg)

            # ---------- main schedule ----------
            for u in ph1_units(0):
                u()
            # zero the sim psum banks once so trimmed regions never hold
            # unbounded garbage (exp of it must stay finite)
            for i in range(2):
                sp0 = sim_ps.tile([128, 2, IC], F32, tag="sp", name="sp0")
                nc.vector.memset(sp0, 0.0)
            feed = ph1_units(1)
            # phase 2: all of b0 (absorbing ph1(b1) between groups), then b1
            for b in range(B):
                if b == 1:
                    while feed:
                        feed.pop(0)()
                for ic in range(NIC):
                    Bc = load_bias_chunk(ic)
                    ph2_chunk(b, ic, Bc, feed if b == 0 else [])
    nc.compile()
    return nc


def _build_launch_b():
    import concourse.bass as bass
    import concourse.tile as tile
    from concourse import bacc, mybir

    F32 = mybir.dt.float32
    F32R = mybir.dt.float32r
    BF16 = mybir.dt.bfloat16

    nc = bacc.Bacc(None)
    at_d = nc.declare_dram_parameter("a_t", [DIM, IC], BF16, isOutput=False)
    s_d = nc.declare_dram_parameter("s_slice", [HEADS, IC], F32, isOutput=False)
    sel_d = nc.declare_dram_parameter("sel", [NCT, HEADS, 128], F32, isOutput=False)
    wo_d = nc.declare_dram_parameter("wo", [DIM, DIM], BF16, isOutput=False)
    out_d = nc.declare_dram_parameter("out_rows", [IC, DIM], BF16, isOutput=True)

    with tile.TileContext(nc) as tc:
        with tc.tile_pool(name="sb", bufs=1) as sb, \
             tc.tile_pool(name="wl", bufs=2) as wl, \
             tc.tile_pool(name="ob", bufs=2) as ob, \
             tc.tile_pool(name="rb_ps", bufs=2, space="PSUM") as rb_ps, \
             tc.tile_pool(name="ps", bufs=2, space="PSUM") as ps:
            a_sb = sb.tile([128, NCT, IC], BF16, tag="a")
            nc.sync.dma_start(
                out=a_sb,
                in_=at_d.ap().rearrange("(t p) i -> p t i", p=128))
            s_sb = sb.tile([HEADS, IC], F32, tag="s")
            nc.sync.dma_start(out=s_sb, in_=s_d.ap())
            sel_sb = sb.tile([HEADS, NCT, 128], F32, tag="sel")
            nc.sync.dma_start(out=sel_sb, in_=sel_d.ap().rearrange("t h p -> h t p"))
            rs_f = sb.tile([HEADS, IC], F32, tag="rs_f")
            nc.vector.reciprocal_approx_fast(out=rs_f, in_=s_sb)
            rs_r = sb.tile([HEADS, IC], F32R, tag="rs_r")
            nc.vector.tensor_copy(out=rs_r, in_=rs_f)
            wo_sb = sb.tile([128, NCT, DIM], BF16, tag="wo")
            nc.sync.dma_start(
                out=wo_sb,
                in_=wo_d.ap().rearrange("(t p) o -> p t o", p=128))
            # normalized bf16 activations: a_n[c, i] = a[c, i] / s[head(c), i]
            a_n = sb.tile([128, NCT, IC], BF16, tag="a_n")
            for ct in range(NCT):
                selr = wl.tile([HEADS, 128], F32R, tag="selr")
                nc.vector.tensor_copy(out=selr, in_=sel_sb[:, ct, :])
                rsb = rb_ps.tile([128, IC], F32, tag="rsb")
                nc.tensor.matmul(rsb, selr, rs_r, start=True, stop=True)
                nc.vector.tensor_mul(a_n[:, ct, :], rsb, a_sb[:, ct, :])
            for m in range(IC // 128):
                osb = ob.tile([128, DIM], BF16, tag="osb")
                for oc in range(2):
                    pp = ps.tile([128, 512], F32, tag="pp")
                    for ct in range(NCT):
                        nc.tensor.matmul(
                            pp, a_n[:, ct, m * 128:(m + 1) * 128],
                            wo_sb[:, ct, oc * 512:(oc + 1) * 512],
                            start=(ct == 0), stop=(ct == NCT - 1))
                    nc.vector.tensor_copy(out=osb[:, oc * 512:(oc + 1) * 512], in_=pp)
                nc.sync.dma_start(out=out_d.ap()[m * 128:(m + 1) * 128, :], in_=osb)

    nc.compile()
    return nc


PROFILE = {"enabled": False, "a_ns": None, "b_ns": None}


def _install_profile_hook():
    """Register the axon NTFF profile hook (the image's antenv lacks
    axon_hooks, so run_bass_kernel_spmd(trace=True) would silently skip
    tracing).  Replicates trn_boot's ctypes recipe."""
    import sys, types, ctypes, contextlib

    if "antenv.axon_hooks" in sys.modules:
        return
    lib = ctypes.CDLL("/opt/axon/libaxon_pjrt.so")
    if not hasattr(lib, "axon_start_nrt_profile"):
        return
    lib.axon_start_nrt_profile.argtypes = [ctypes.POINTER(ctypes.c_int64), ctypes.c_size_t]
    lib.axon_start_nrt_profile.restype = ctypes.c_int64
    lib.axon_stop_nrt_profile.argtypes = [ctypes.c_char_p]
    lib.axon_stop_nrt_profile.restype = ctypes.c_int64

    @contextlib.contextmanager
    def _hook(output_dir, device_ids):
        import jax
        jax.devices()
        if device_ids:
            ids = (ctypes.c_int64 * len(device_ids))(*device_ids)
            rc = lib.axon_start_nrt_profile(ids, len(device_ids))
        else:
            rc = lib.axon_start_nrt_profile(None, 0)
        if rc != 0:
            raise RuntimeError(f"axon_start_nrt_profile rc={rc}")
        try:
            yield
        finally:
            n = lib.axon_stop_nrt_profile(str(output_dir).encode())
            print(f"profile: {n} file(s) written to {output_dir}")

    mod = types.ModuleType("antenv.axon_hooks")
    mod.get_axon_ntff_profile_hook = lambda: _hook
    mod.set_axon_ntff_profile_hook = lambda h: None
    sys.modules["antenv.axon_hooks"] = mod

    # avoid the S3 artifact upload inside the trace path
    from concourse import bass_utils
    bass_utils.upload_artifacts = lambda tmpdir: ""


def kernel(x, gamma, Wq, Wkv, q_scale, k_scale, Wo, rel_pos_bias, mask):
    from concourse.bass_utils import run_bass_kernel_spmd
    import ml_dtypes

    x = np.ascontiguousarray(np.asarray(x, dtype=np.float32))
    gamma = np.asarray(gamma, dtype=np.float32)
    Wq = np.asarray(Wq, dtype=np.float32)
    Wkv = np.asarray(Wkv, dtype=np.float32)
    q_scale = np.asarray(q_scale, dtype=np.float32)
    k_scale = np.asarray(k_scale, dtype=np.float32)
    Wo = np.ascontiguousarray(np.asarray(Wo, dtype=np.float32))
    rel_pos_bias = np.asarray(rel_pos_bias, dtype=np.float32)
    mask = np.asarray(mask)
    mask_ones = bool(mask.all())

    if PROFILE["enabled"]:
        _install_profile_hook()
    akey = ("a", mask_ones)
    if akey not in _cache:
        _cache[akey] = _build_launch_a(mask_ones)
    if "b" not in _cache:
        _cache["b"] = _build_launch_b()

    BF = ml_dtypes.bfloat16
    F8 = ml_dtypes.float8_e4m3fn
    # host-side prep: LN stats; rstd is folded into x (it cancels in the q/k
    # l2norm and is exactly what v needs), gamma into the weights.  All large
    # tensors are laid out so device DMAs are partition-major contiguous.
    mu = x.mean(-1)
    var = x.var(-1)
    rstd = 1.0 / np.sqrt(var + LN_EPS)                         # [B, N]
    xh = x * rstd[:, :, None]
    xT = xh.transpose(0, 2, 1)                                 # [B, DIM, N]
    XR = np.ascontiguousarray(
        xT.reshape(B, NCT, 128, NIC, IC).transpose(0, 3, 2, 1, 4)).astype(BF)
    numu = (-mu * rstd).astype(BF)                             # [B, N]
    kb = np.where(mask, 0.0, NEG).astype(np.float32)

    wq_f = gamma[:, None] * Wq
    wk_f = gamma[:, None] * Wkv[:, :DIM]
    wv_f = gamma[:, None] * Wkv[:, DIM:]

    sblk4q = np.zeros((4, 128), np.float32)
    sblk4q[0, 0:64] = q_scale * 8.0
    sblk4q[1, 64:128] = q_scale * 8.0
    sblk4k = np.zeros((4, 128), np.float32)
    sblk4k[2, 0:64] = k_scale
    sblk4k[3, 64:128] = k_scale

    # B = exp(rel_pos_bias^T) in bf16, per-chunk [p, jt, h, i] layout
    rpbT = rel_pos_bias.transpose(0, 2, 1)                     # [H, j, i]
    BE = np.exp(rpbT)

    def wlayout(w):
        # [DIM, EH] -> [128, NCT, EH] partition-major
        return np.ascontiguousarray(w.reshape(NCT, 128, EH).transpose(1, 0, 2))

    in_maps_a = []
    for c in range(NCORES):
        es = slice(EH * c, EH * (c + 1))
        wq_s = wlayout(wq_f[:, es]).astype(BF)
        wk_s = wlayout(wk_f[:, es]).astype(BF)
        wv_s = wlayout(wv_f[:, es]).astype(BF)
        # [h, jt, p, ic, i]
        bcore = BE[2 * c:2 * c + 2].reshape(2, NJT, 128, NIC, IC)
        m = {
            "xr": XR,
            "wq": wq_s, "wk": wk_s, "wv": wv_s,
            "csq": wq_s.astype(np.float32).sum((0, 1))[None, :].astype(BF),
            "csk": wk_s.astype(np.float32).sum((0, 1))[None, :].astype(BF),
            "csv": wv_s.astype(np.float32).sum((0, 1))[None, :].astype(BF),
            "numu": numu,
            "sblk4q": sblk4q, "sblk4k": sblk4k,
        }
        for ic in range(NIC):
            jmax = 4 * (ic + 1)
            m[f"bc{ic}"] = np.ascontiguousarray(
                bcore[:, 0:jmax, :, ic, :].transpose(2, 1, 0, 3)).astype(BF)
        if not mask_ones:
            m["kb"] = kb
        in_maps_a.append(m)
    res_a = run_bass_kernel_spmd(_cache[akey], in_maps_a, list(range(NCORES)),
                                 trace=PROFILE["enabled"])
    if PROFILE["enabled"]:
        PROFILE["a_ns"] = res_a.exec_time_ns

    AT = np.empty((B, DIM, N), np.float32)
    S = np.empty((B, HEADS, N), np.float32)
    for c in range(NCORES):
        ao = res_a.results[c]["at_out"]            # [B, 2, 65, N]
        for h in range(2):
            AT[:, EH * c + 64 * h:EH * c + 64 * h + 64, :] = ao[:, h, 0:64, :]
            S[:, 2 * c + h, :] = ao[:, h, 64, :]
    AT_bf = AT.astype(BF)
    Wo_bf = Wo.astype(BF)

    sel = np.zeros((NCT, HEADS, 128), np.float32)
    for ct in range(NCT):
        sel[ct, 2 * ct, 0:64] = 1.0
        sel[ct, 2 * ct + 1, 64:128] = 1.0

    in_maps_b = []
    for c in range(NCORES):
        bi, ic = c // NIC, c % NIC
        in_maps_b.append({
            "a_t": np.ascontiguousarray(AT_bf[bi][:, ic * IC:(ic + 1) * IC]),
            "s_slice": np.ascontiguousarray(S[bi][:, ic * IC:(ic + 1) * IC]),
            "sel": sel,
            "wo": Wo_bf,
        })
    res_b = run_bass_kernel_spmd(_cache["b"], in_maps_b, list(range(NCORES)),
                                 trace=PROFILE["enabled"])
    if PROFILE["enabled"]:
        PROFILE["b_ns"] = res_b.exec_time_ns

    out = np.empty((B, N, DIM), np.float32)
    for c in range(NCORES):
        bi, ic = c // NIC, c % NIC
        out[bi, ic * IC:(ic + 1) * IC, :] = res_b.results[c]["out_rows"].astype(np.float32)
    return out
